# revision 19
# baseline (speedup 1.0000x reference)
"""Gated attention-with-pair-bias kernel for 8 Trainium2 NeuronCores.

Problem: B=2, Q=K=2048, C=256, H=8 heads, D=32 per head.
  q = (q_x @ Wq.T)/sqrt(D); k = kv_x @ Wk.T; v = kv_x @ Wv.T   (per head h)
  S = q @ k.T + bias_mask + bias_pair; w = softmax_k(S)
  o = (w @ v) * sigmoid(q_x @ Wg.T + bg); out = o @ Wo.T + bo

Sharding: one head per core; each core does both batch elements so each
head's bias slice is read from HBM exactly once.

v2 design (vs v1): the PE no longer injects bias_pair via identity
matmuls.  exp(s + bp) = exp(s) * exp(bp): the host precomputes
exp(bias_pair) in bf16 and the DVE multiplies it into the exp output
(bf16 x bf16 tensor_tensor, 2 elem/cycle/lane).  Score matmuls are now
single start/stop ops in 2 concurrent 32-row PE bands (no PSUM RMW),
and both k-tiles of a group accumulate o into the SAME po partitions
0:33 (32 o dims + the softmax-denominator ones column), so the old
97-row even/odd merge matmul is gone; only a 1-row ones matmul remains
to broadcast l across partitions for the division.
bias_mask folds into v multiplicatively (exp(bm) row scaling applied
during the PSUM->SBUF evacuation of v, ones column included).
kvT is stored with k-tiles permuted (pos 4j+g holds tile 4g+j) so the
k.T strip matmuls stream contiguous SBUF.
The gate uses tanh (same ScalarE table as Exp -> zero table reloads):
sigmoid(z) = (tanh(z/2)+1)/2, the 1/2 folded into Wv.
The division by l commutes past the output projection: og = (gate
combo) * o * (1/l) on [32,512] tiles (GpSimd), then Wo matmul, copy out.
A knob (DVE_EXP_TILES) can move some exp tiles from ScalarE to a custom
DVE op (EXP2_BITS_ANT) that builds the bf16 BIT PATTERN of 2^(y/128)
arithmetically: round/frac split via the +1.5*2^30 magic constant, a
deg-2 mantissa polynomial, and an int16 store whose bits are the bf16
weight.  Both paths share one uniform scale 2^c that cancels in o/l.
"""

import math
import os
import sys

sys.path.insert(0, "/opt/trn_rl_repo")

import numpy as np

H, D, B, Q, K, C = 8, 32, 2, 2048, 2048, 256
NQC = 4          # q chunks of 512
NKT = K // 128   # 16 k tiles

# exp2-bit-construction constants (fit offline)
MAGIC = 1.5 * 2 ** 30
EXP_A = 0.00255          # t^2 coeff (2^7-scaled domain)
EXP_B = 0.996            # t coeff
ALPHA = 53.7             # mantissa-poly constant, added post-round
BSHIFT = 16256.0         # 127*128: bf16 exponent bias in 2^7 units
C_CENTER = 0.5033798133168   # log2(w) - y/128 (uniform; cancels in o/l)
LOG2E = math.log2(math.e)
SCAL_SCALE = math.log(2.0) / 128.0
SCAL_BIAS = C_CENTER * math.log(2.0)

# per-(qc,b) psum-tile indices (2g+X) whose exp runs on the DVE custom op
# instead of ScalarE.  Tune for engine balance.
_dve_env = os.environ.get("DVE_EXP_TILES", "")
DVE_TILES = frozenset(int(x) for x in _dve_env.split(",") if x != "")

_CACHE = {}


def _register_exp2_op():
    """Register the custom DVE op (in-process; documented extension API)."""
    import concourse.dve_ops as dvo
    from concourse.dve_spec import Spec, Src0, Src1, C0, C1, C2, lower
    from concourse.dve_uop import DveOpSpec

    if "EXP2_BITS_ANT" in dvo._SUB_OPCODE_FOR_NAME:
        return next(o for o in dvo.OPS if o.name == "EXP2_BITS_ANT")

    m = Src0 + C0
    r = m - C0
    t = Src0 - r
    h3 = ((t * C1) + C2) * t
    body = (h3 + r) + Src1

    def ref(in0, in1, s0, s1, imm2):
        f32 = np.float32
        mm = (in0.astype(f32) + f32(s0)).astype(f32)
        rr = (mm - f32(s0)).astype(f32)
        tt = (in0.astype(f32) - rr).astype(f32)
        hh = (((tt * f32(s1)).astype(f32) + f32(imm2)).astype(f32) * tt).astype(f32)
        return ((hh + rr).astype(f32) + in1.astype(f32)).astype(f32)

    spec = Spec(body=body, reference=ref)
    row = dvo._CUSTOM_DVE_ROW_BASE + len(dvo.OPS)
    assert row < 0x20
    shas = {v: DveOpSpec(name="EXP2_BITS_ANT", opcode=row,
                         uops=lower(spec, ver=v), rd1_en=True).sha(v)
            for v in ("v3", "v4")}
    op = dvo.DveOp("EXP2_BITS_ANT", spec, subdim=False, uops_sha=shas)
    dvo.OPS.append(op)
    dvo._SUB_OPCODE_FOR_NAME[op.name] = row
    dvo.CUSTOM_DVE_SPECS[op.name] = spec
    op.compile("v3")
    return op


def _build():
    import concourse.bacc as bacc
    import concourse.mybir as mybir
    from concourse.tile import TileContext

    EXP2 = _register_exp2_op()

    F32 = mybir.dt.float32
    F32R = mybir.dt.float32r
    BF16 = mybir.dt.bfloat16
    FP16 = mybir.dt.float16
    I16 = mybir.dt.int16
    EXP = mybir.ActivationFunctionType.Exp
    TANH = mybir.ActivationFunctionType.Tanh
    COPY = mybir.ActivationFunctionType.Copy
    MULT = mybir.AluOpType.mult

    kdebug = bool(os.environ.get("KDEBUG"))
    nc = bacc.Bacc(None, target_bir_lowering=False)
    qxT = nc.dram_tensor("qxT", [B, 2, 128, Q], BF16, kind="ExternalInput")
    kvT = nc.dram_tensor("kvT", [B, 2, 128, K], BF16, kind="ExternalInput")
    ebp = nc.dram_tensor("ebp", [NQC, 128, NKT, 512], BF16, kind="ExternalInput")
    expbm = nc.dram_tensor("expbm", [B, 128, NKT], F32, kind="ExternalInput")
    wq = nc.dram_tensor("wq", [2, 128, 128], BF16, kind="ExternalInput")
    wkvg = nc.dram_tensor("wkvg", [2, 128, 3 * D], BF16, kind="ExternalInput")
    wobg = nc.dram_tensor("wobg", [D, C + 1], F32R, kind="ExternalInput")
    outT = nc.dram_tensor("outT", [B, 2, 128, Q], FP16, kind="ExternalOutput")
    if kdebug:
        dbg_gp = nc.dram_tensor("dbg_gp", [32, Q], F32, kind="ExternalOutput")
        dbg_po = nc.dram_tensor("dbg_po", [D + 1, 512], F32, kind="ExternalOutput")
        dbg_wt = nc.dram_tensor("dbg_wt", [2, 128, 1024], BF16, kind="ExternalOutput")
        dbg_kq = nc.dram_tensor("dbg_kq", [128, 512 + Q], BF16, kind="ExternalOutput")
        dbg_vt = nc.dram_tensor("dbg_vt", [128, NKT * (D + 1)], BF16, kind="ExternalOutput")
        dbg_ax = nc.dram_tensor("dbg_ax", [4, 128, K], BF16, kind="ExternalOutput")
        dbg_eb = nc.dram_tensor("dbg_eb", [128, NKT, 512], BF16, kind="ExternalOutput")

    with TileContext(nc) as tc:
        with (
            tc.tile_pool(name="ld", bufs=1) as ld,
            tc.tile_pool(name="pers", bufs=1) as pers,
            tc.tile_pool(name="w0p", bufs=4) as w0pool,
            tc.tile_pool(name="wp", bufs=6) as wpool,
            tc.tile_pool(name="ep", bufs=2) as epool,
            tc.tile_pool(name="ob", bufs=3) as obpool,
            tc.tile_pool(name="ps_sc", bufs=2, space="PSUM") as ps_sc,
            tc.tile_pool(name="ps_o", bufs=2, space="PSUM") as ps_o,
            tc.tile_pool(name="ps_m", bufs=2, space="PSUM") as ps_m,
        ):
            # ---- weights first (small), then per-batch activations ----
            wkvg_sb, wq_sb = [], []
            for ch in range(2):
                t = pers.tile([128, 3 * D], BF16, name=f"wkvg_sb{ch}")
                nc.sync.dma_start(out=t[:, :], in_=wkvg[ch, :, :])
                wkvg_sb.append(t)
            wk_sb = [t[:, 0:D] for t in wkvg_sb]
            wv_sb = [t[:, D:2 * D] for t in wkvg_sb]
            wg_sb = [t[:, 2 * D:3 * D] for t in wkvg_sb]

            kv_all, qx_all = {}, {}

            def load_acts(b):
                for ch in range(2):
                    t = ld.tile([128, K], BF16, name=f"kv{b}{ch}", tag=f"kv{b}{ch}")
                    nc.sync.dma_start(out=t[:, :], in_=kvT[b, ch, :, :])
                    kv_all[(b, ch)] = t
                for ch in range(2):
                    t = ld.tile([128, Q], BF16, name=f"qx{b}{ch}", tag=f"qx{b}{ch}")
                    nc.sync.dma_start(out=t[:, :], in_=qxT[b, ch, :, :])
                    qx_all[(b, ch)] = t

            load_acts(0)
            for ch in range(2):
                t = pers.tile([128, 128], BF16, name=f"wq_sb{ch}")
                nc.sync.dma_start(out=t[:, :], in_=wq[ch, :, :])
                wq_sb.append(t)

            # small consts
            bm_sb = []
            for b in range(B):
                t = pers.tile([128, NKT], F32, name=f"bm_sb{b}")
                nc.sync.dma_start(out=t[:, :], in_=expbm[b, :, :])
                bm_sb.append(t)
            wobg_sb = pers.tile([D, C + 1], F32R, name="wobg_sb")
            nc.sync.dma_start(out=wobg_sb[:, :], in_=wobg[:, :])
            wo_sb = wobg_sb[:, 0:C]
            bg_sb = wobg_sb[:, C:C + 1].bitcast(F32)
            al_sb = pers.tile([128, 1024], F32, name="al_sb")
            nc.vector.memset(al_sb[:, :], BSHIFT + ALPHA)
            sb_sb = pers.tile([128, 1], F32, name="sb_sb")
            nc.gpsimd.memset(sb_sb[:, :], SCAL_BIAS)
            # ones row at partition 32 (matmul wants lhsT/rhs base aligned:
            # the moving l row lives at partition 32 of posb)
            ones32_sb = pers.tile([D + 1, D], F32, name="ones32_sb")
            nc.gpsimd.memset(ones32_sb[:, :], 1.0)

            # exp(bias_pair) preload: whole head slice in SBUF (64KB/part)
            ebp_sb = pers.tile([128, NQC, NKT * 512], BF16, name="ebp_sb")
            ebp_loaded = set()

            def load_ebp(qc):
                if qc in ebp_loaded:
                    return
                ebp_loaded.add(qc)
                nc.sync.dma_start(out=ebp_sb[:, qc, :], in_=ebp[qc, :, :, :]
                                  .rearrange("p t q -> p (t q)"))

            load_ebp(0)
            load_ebp(1)
            load_acts(1)
            load_ebp(2)
            load_ebp(3)

            # ---- per-batch projections ----
            qT_rep, kT_sb, v_sb, gp_sb = {}, {}, {}, {}

            def emit_prologue(b):
                qx_b = [qx_all[(b, ch)] for ch in range(2)]
                kv_b = [kv_all[(b, ch)] for ch in range(2)]
                qT = pers.tile([128, Q], BF16, name=f"qT{b}")
                gp1 = pers.tile([32, Q], F32, name=f"gp1{b}")
                kT = pers.tile([128, 512], BF16, name=f"kT{b}")
                vt = pers.tile([128, NKT, D + 1], BF16, name=f"v{b}")
                qT_rep[b] = qT; kT_sb[b] = kT; v_sb[b] = vt; gp_sb[b] = gp1

                # k.T in strip layout: strip j (partitions 32j) holds tiles
                # {4g+j} at free cols g*128.  kvT is host-permuted so pos
                # 4j+g holds tile 4g+j -> moving data is contiguous.
                ps = ps_m.tile([128, 512], F32, tag="m", name=f"pk{b}")
                for ch in range(2):
                    for j in range(4):
                        nc.tensor.matmul(
                            ps[32 * j:32 * j + 32, :], wk_sb[ch][:, :],
                            kv_b[ch][:, j * 512:(j + 1) * 512],
                            start=(ch == 0), stop=(ch == 1),
                            tile_position=(0, 32 * j))
                nc.vector.tensor_copy(kT[:, :], ps[:, :])

                # v tiles [k-partitions, d] + exp(bm) ones col; the exp(bm)
                # row scale (bias_mask fold: w and l scale alike) is applied
                # during PSUM evacuation via tensor_scalar_mul.
                for p in range(8):
                    ps = ps_m.tile([128, 2, D], F32, tag="m", name=f"pv{b}{p}",
                                   padded_shape=[128, 2, 256])
                    for i in range(2):
                        kt = 2 * p + i
                        pos = 4 * (kt % 4) + kt // 4
                        for ch in range(2):
                            nc.tensor.matmul(
                                ps[:, i, :],
                                kv_b[ch][:, pos * 128:(pos + 1) * 128],
                                wv_sb[ch][:, :], start=(ch == 0), stop=(ch == 1))
                    for i in range(2):
                        kt = 2 * p + i
                        nc.vector.tensor_scalar_mul(
                            vt[:, kt, 0:D], ps[:, i, :],
                            bm_sb[b][:, kt:kt + 1])
                # ones columns (scaled by exp(bm)) for the denominator
                nc.vector.tensor_copy(
                    vt[:, :, D:D + 1].rearrange("p k o -> p (k o)"),
                    bm_sb[b][:, :])

                # q.T replicated into 4 partition strips via 4x-duplicated
                # weight columns (host-prepared); scale folded: sD*log2e*128
                for qc in range(NQC):
                    ps = ps_m.tile([128, 512], F32, tag="m", name=f"pq{b}{qc}")
                    for ch in range(2):
                        nc.tensor.matmul(
                            ps[:, :], wq_sb[ch][:, :],
                            qx_b[ch][:, qc * 512:(qc + 1) * 512],
                            start=(ch == 0), stop=(ch == 1))
                    nc.vector.tensor_copy(qT[:, qc * 512:(qc + 1) * 512], ps[:, :])

                # gate: tanh(z/2 + bg/2); (tanh+1)/2 with the 1/2 in Wv
                for qc in range(NQC):
                    ps = ps_m.tile([32, 512], F32, tag="m", name=f"pg{b}{qc}",
                                   padded_shape=[128, 512])
                    for ch in range(2):
                        nc.tensor.matmul(
                            ps[:, :], wg_sb[ch][:, :],
                            qx_b[ch][:, qc * 512:(qc + 1) * 512],
                            start=(ch == 0), stop=(ch == 1))
                    gt = epool.tile([32, 512], F32, tag="gt", name=f"gt{b}{qc}")
                    nc.scalar.activation(gt[:, :], ps[:, :],
                                         TANH, bias=bg_sb, scale=0.5)
                    nc.vector.tensor_scalar_add(
                        gp1[:, qc * 512:(qc + 1) * 512], gt[:, :], 1.0)

            emit_prologue(0)

            # ---- main attention loop (b outer: b1 acts can arrive late) ----
            from collections import deque
            workq = deque()

            def make_ep(qc, b, po):
                # po rows 0:32 = o (all 16 k-tiles), row 32 = l.  Evacuate
                # once; broadcast l across partitions via the ones matmul;
                # gate+normalize on [32,512].  Split in two so the PE-side
                # Wo matmuls trail the GpSimd gating chain by a tile.
                st = {}

                def ep_part1():
                    posb = epool.tile([D + 1, 512], F32R, tag="posb",
                                      name=f"posb{qc}{b}")
                    nc.scalar.activation(posb[:, :], po[0:D + 1, :], COPY)
                    if kdebug and b == 0 and qc == 0:
                        for ch in range(2):
                            nc.sync.dma_start(
                                out=dbg_ax[ch], in_=kv_all[(0, ch)][:, :])
                            nc.sync.dma_start(
                                out=dbg_ax[2 + ch], in_=qx_all[(0, ch)][:, :])
                        nc.sync.dma_start(out=dbg_eb[:, :, :],
                                          in_=ebp_sb[:, 0, :]
                                          .rearrange("p (t q) -> p t q", q=512))
                        nc.sync.dma_start(out=dbg_po[:, :],
                                          in_=posb[:, :].bitcast(F32))
                        nc.sync.dma_start(out=dbg_gp[:, :], in_=gp_sb[0][:, :])
                        nc.sync.dma_start(out=dbg_kq[:, 0:512],
                                          in_=kT_sb[0][:, :])
                        nc.sync.dma_start(out=dbg_kq[:, 512:],
                                          in_=qT_rep[0][:, :])
                        nc.sync.dma_start(
                            out=dbg_vt[:, :],
                            in_=v_sb[0][:, :, :].rearrange("p k o -> p (k o)"))
                    psl = ps_m.tile([D, 512], F32, tag="m",
                                    name=f"psl{qc}{b}",
                                    padded_shape=[128, 512])
                    nc.tensor.matmul(
                        psl[:, :], ones32_sb[D:D + 1, :].bitcast(F32R),
                        posb[D:D + 1, :], start=True, stop=True)
                    rlt = epool.tile([D, 512], F32, tag="rl",
                                     name=f"rl{qc}{b}")
                    nc.vector.reciprocal_approx_fast(rlt[:, :], psl[:, :])
                    og = epool.tile([D, 512], F32R, tag="og",
                                    name=f"og{qc}{b}")
                    nc.gpsimd.tensor_tensor(
                        og[:, :], gp_sb[b][:, qc * 512:(qc + 1) * 512],
                        posb[0:D, :], op=MULT)
                    og2 = epool.tile([D, 512], F32R, tag="og2",
                                     name=f"og2{qc}{b}")
                    nc.gpsimd.tensor_tensor(og2[:, :], og[:, :], rlt[:, :],
                                            op=MULT)
                    st["og2"] = og2

                def ep_part2():
                    og2 = st["og2"]
                    for half in range(2):
                        pp = ps_m.tile([128, 512], F32, tag="m",
                                       name=f"pp{qc}{b}{half}")
                        nc.tensor.matmul(
                            pp[:, :], wo_sb[:, half * 128:(half + 1) * 128],
                            og2[:, :], start=True, stop=True)
                        ot = obpool.tile([128, 512], FP16, tag="ot",
                                         name=f"ot{qc}{b}{half}")
                        if half == 0:
                            nc.vector.tensor_copy(ot[:, :], pp[:, :])
                        else:
                            nc.scalar.activation(ot[:, :], pp[:, :], COPY)
                        nc.sync.dma_start(
                            out=outT[b, half, :, qc * 512:(qc + 1) * 512],
                            in_=ot[:, :])

                return ep_part1, ep_part2

            for b in range(B):
                if b == 1:
                    emit_prologue(1)
                for qc in range(NQC):
                    po = ps_o.tile([128, 512], F32, tag="o", name=f"po{qc}{b}")
                    for g in range(4):
                        for X in range(2):
                            tix = 2 * g + X
                            dve = tix in DVE_TILES
                            sc = ps_sc.tile([128, 1024], F32, tag="sc",
                                            name=f"s{qc}{b}{g}{X}")
                            for jj in range(2):
                                j = 2 * X + jj
                                nc.tensor.matmul(
                                    sc[:, jj * 512:(jj + 1) * 512],
                                    kT_sb[b][32 * j:32 * j + 32,
                                             g * 128:(g + 1) * 128],
                                    qT_rep[b][32 * j:32 * j + 32,
                                              qc * 512:(qc + 1) * 512],
                                    start=True, stop=True,
                                    tile_position=(32 * j, 0))
                            wt0 = w0pool.tile([128, 1024], BF16, tag="w0",
                                              name=f"w0{qc}{b}{g}{X}")
                            if dve:
                                nc.vector._custom_dve(
                                    EXP2, out=wt0[:, :].bitcast(I16),
                                    in0=sc[:, :], in1=al_sb[:, :],
                                    s0=MAGIC, s1=EXP_A, imm2=EXP_B)
                            else:
                                nc.scalar.activation(wt0[:, :], sc[:, :],
                                                     EXP, bias=sb_sb[:, :],
                                                     scale=SCAL_SCALE)
                            wt = wpool.tile([128, 1024], BF16, tag="w",
                                            name=f"w{qc}{b}{g}{X}")
                            ks = (4 * g + 2 * X) * 512
                            nc.vector.tensor_tensor(
                                wt[:, :], wt0[:, :],
                                ebp_sb[:, qc, ks:ks + 1024], op=MULT)
                            if kdebug and b == 0 and qc == 0 and tix == 0:
                                nc.sync.dma_start(out=dbg_wt[0], in_=wt0[:, :])
                                nc.sync.dma_start(out=dbg_wt[1], in_=wt[:, :])

                            def make_o(b, g, X, po, wt):
                                p = 2 * g + X

                                def emit_o():
                                    ktA = 4 * g + 2 * X
                                    nc.tensor.matmul(
                                        po[0:D + 1, :], v_sb[b][:, ktA, :],
                                        wt[:, 0:512], start=(p == 0),
                                        stop=False)
                                    nc.tensor.matmul(
                                        po[0:D + 1, :], v_sb[b][:, ktA + 1, :],
                                        wt[:, 512:1024], start=False,
                                        stop=(p == 7))
                                return emit_o
                            workq.append(make_o(b, g, X, po, wt))
                            if tix == 7:
                                ep1, ep2 = make_ep(qc, b, po)
                                workq.append(ep1)
                                workq.append(ep2)
                            # o-matmuls (and split epilogues) trail by ~2
                            # tiles so the in-order PE queue never waits on
                            # the exp/mult engines or the gating chain
                            while len(workq) > 2:
                                workq.popleft()()

            while workq:
                workq.popleft()()
    nc.compile()
    return nc


def _get_nc():
    if "nc" not in _CACHE:
        _CACHE["nc"] = _build()
    return _CACHE["nc"]


def kernel(q_x, kv_x, bias_mask, bias_pair, Wq, Wk, Wv, Wo, bo, Wg, bg):
    from concourse.bass_utils import run_bass_kernel_spmd

    nc = _get_nc()
    f32 = np.float32
    q_x = np.asarray(q_x, f32); kv_x = np.asarray(kv_x, f32)
    bias_mask = np.asarray(bias_mask, f32); bias_pair = np.asarray(bias_pair, f32)
    Wq = np.asarray(Wq, f32); Wk = np.asarray(Wk, f32); Wv = np.asarray(Wv, f32)
    Wo = np.asarray(Wo, f32); bo = np.asarray(bo, f32); Wg = np.asarray(Wg, f32)
    bg = np.asarray(bg, f32)

    import ml_dtypes
    _bf16 = ml_dtypes.bfloat16
    sD = 1.0 / math.sqrt(D)
    yscale = LOG2E * 128.0
    qxT_dev = np.ascontiguousarray(
        q_x.transpose(0, 2, 1).reshape(B, 2, 128, Q)).astype(_bf16)
    # kvT with k-tiles permuted: position 4j+g holds tile 4g+j, so the
    # k.T strip matmuls read contiguous moving data.
    kv_t = kv_x.transpose(0, 2, 1).reshape(B, 2, 128, NKT, 128)
    perm = [4 * (p % 4) + p // 4 for p in range(NKT)]
    kvT_dev = np.ascontiguousarray(
        kv_t[:, :, :, perm, :].reshape(B, 2, 128, K)).astype(_bf16)
    bm_dev = np.ascontiguousarray(
        np.exp(bias_mask.reshape(B, NKT, 128).transpose(0, 2, 1)))

    def wsplit(W, h, scale=1.0):
        # [2, 128, D] view of (W_h * scale).T with W_h = W[h*D:(h+1)*D, :]
        return np.ascontiguousarray(
            (W[h * D:(h + 1) * D, :] * scale).T.reshape(2, 128, D))

    def wrep(W, h, scale=1.0):
        # weight columns duplicated 4x -> M=128 matmul emits 4 replicas
        wt = wsplit(W, h, scale)                       # [2, 128, D]
        return np.ascontiguousarray(np.tile(wt, (1, 1, 4)))

    in_maps = []
    for h in range(H):
        bp = bias_pair[0, h].T                                 # [K, Q]
        ebp = np.exp(bp).astype(_bf16)
        ebp_dev = np.ascontiguousarray(
            ebp.reshape(NKT, 128, NQC, 512).transpose(2, 1, 0, 3))
        wkvg_h = np.concatenate(
            [wsplit(Wk, h), wsplit(Wv, h, 0.5), wsplit(Wg, h)],
            axis=2).astype(_bf16)
        wobg_h = np.concatenate(
            [np.ascontiguousarray(Wo[:, h * D:(h + 1) * D].T),
             0.5 * bg[h * D:(h + 1) * D, None]], axis=1)
        in_maps.append({
            "qxT": qxT_dev, "kvT": kvT_dev,
            "ebp": ebp_dev,
            "expbm": bm_dev,
            "wq": wrep(Wq, h, sD * yscale).astype(_bf16),
            "wkvg": np.ascontiguousarray(wkvg_h),
            "wobg": np.ascontiguousarray(wobg_h.astype(f32)),
        })

    try:
        res = run_bass_kernel_spmd(nc, in_maps, core_ids=list(range(H)))
    except Exception:
        # rare transient accelerator fault — one retry after a short pause
        import time as _time
        _time.sleep(5)
        res = run_bass_kernel_spmd(nc, in_maps, core_ids=list(range(H)))
    out = np.zeros((B, Q, C), f32)
    for h in range(H):
        p = res.results[h]["outT"].astype(f32).reshape(B, C, Q)
        out += p.transpose(0, 2, 1)
    out += bo
    return out


# revision 28
# speedup vs baseline: 1.0284x; 1.0284x over previous
"""Gated attention-with-pair-bias kernel for 8 Trainium2 NeuronCores.

Problem: B=2, Q=K=2048, C=256, H=8 heads, D=32 per head.
  q = (q_x @ Wq.T)/sqrt(D); k = kv_x @ Wk.T; v = kv_x @ Wv.T   (per head h)
  S = q @ k.T + bias_mask + bias_pair; w = softmax_k(S)
  o = (w @ v) * sigmoid(q_x @ Wg.T + bg); out = o @ Wo.T + bo

Sharding: one head per core; each core does both batch elements so each
head's bias slice is read from HBM exactly once.

v2 design (vs v1): the PE no longer injects bias_pair via identity
matmuls.  exp(s + bp) = exp(s) * exp(bp): the host precomputes
exp(bias_pair) in bf16 and the DVE multiplies it into the exp output
(bf16 x bf16 tensor_tensor, 2 elem/cycle/lane).  Score matmuls are now
single start/stop ops in 2 concurrent 32-row PE bands (no PSUM RMW),
and both k-tiles of a group accumulate o into the SAME po partitions
0:33 (32 o dims + the softmax-denominator ones column), so the old
97-row even/odd merge matmul is gone; only a 1-row ones matmul remains
to broadcast l across partitions for the division.
bias_mask folds into v multiplicatively (exp(bm) row scaling applied
during the PSUM->SBUF evacuation of v, ones column included).
kvT is stored with k-tiles permuted (pos 4j+g holds tile 4g+j) so the
k.T strip matmuls stream contiguous SBUF.
The gate uses tanh (same ScalarE table as Exp -> zero table reloads):
sigmoid(z) = (tanh(z/2)+1)/2, the 1/2 folded into Wv.
The division by l commutes past the output projection: og = (gate
combo) * o * (1/l) on [32,512] tiles (GpSimd), then Wo matmul, copy out.
A knob (DVE_EXP_TILES) can move some exp tiles from ScalarE to a custom
DVE op (EXP2_BITS_ANT) that builds the bf16 BIT PATTERN of 2^(y/128)
arithmetically: round/frac split via the +1.5*2^30 magic constant, a
deg-2 mantissa polynomial, and an int16 store whose bits are the bf16
weight.  Both paths share one uniform scale 2^c that cancels in o/l.
"""

import math
import os
import sys

sys.path.insert(0, "/opt/trn_rl_repo")

import numpy as np

H, D, B, Q, K, C = 8, 32, 2, 2048, 2048, 256
NQC = 4          # q chunks of 512
NKT = K // 128   # 16 k tiles

# exp2-bit-construction constants (fit offline)
MAGIC = 1.5 * 2 ** 30
EXP_A = 0.00255          # t^2 coeff (2^7-scaled domain)
EXP_B = 0.996            # t coeff
ALPHA = 53.7             # mantissa-poly constant, added post-round
BSHIFT = 16256.0         # 127*128: bf16 exponent bias in 2^7 units
C_CENTER = 0.5033798133168   # log2(w) - y/128 (uniform; cancels in o/l)
LOG2E = math.log2(math.e)
SCAL_SCALE = math.log(2.0) / 128.0
SCAL_BIAS = C_CENTER * math.log(2.0)

# per-(qc,b) psum-tile indices (2g+X) whose exp runs on the DVE custom op
# instead of ScalarE.  Tune for engine balance.
_dve_env = os.environ.get("DVE_EXP_TILES", "")
DVE_TILES = frozenset(int(x) for x in _dve_env.split(",") if x != "")
# tile indices whose bias_pair is injected on the PE via identity matmuls
# (PSUM pre-load, v1-style) instead of the DVE exp(bp) multiply.  More id
# tiles -> more PE work but denser PE fill (keeps the HAM clock-gate at
# 8/8) and less DVE work.
_id_env = os.environ.get("ID_TILES", "0,3,6")
ID_TILES = frozenset(int(x) for x in _id_env.split(",") if x != "")

_CACHE = {}


def _register_exp2_op():
    """Register the custom DVE op (in-process; documented extension API)."""
    import concourse.dve_ops as dvo
    from concourse.dve_spec import Spec, Src0, Src1, C0, C1, C2, lower
    from concourse.dve_uop import DveOpSpec

    if "EXP2_BITS_ANT" in dvo._SUB_OPCODE_FOR_NAME:
        return next(o for o in dvo.OPS if o.name == "EXP2_BITS_ANT")

    m = Src0 + C0
    r = m - C0
    t = Src0 - r
    h3 = ((t * C1) + C2) * t
    body = (h3 + r) + Src1

    def ref(in0, in1, s0, s1, imm2):
        f32 = np.float32
        mm = (in0.astype(f32) + f32(s0)).astype(f32)
        rr = (mm - f32(s0)).astype(f32)
        tt = (in0.astype(f32) - rr).astype(f32)
        hh = (((tt * f32(s1)).astype(f32) + f32(imm2)).astype(f32) * tt).astype(f32)
        return ((hh + rr).astype(f32) + in1.astype(f32)).astype(f32)

    spec = Spec(body=body, reference=ref)
    row = dvo._CUSTOM_DVE_ROW_BASE + len(dvo.OPS)
    assert row < 0x20
    shas = {v: DveOpSpec(name="EXP2_BITS_ANT", opcode=row,
                         uops=lower(spec, ver=v), rd1_en=True).sha(v)
            for v in ("v3", "v4")}
    op = dvo.DveOp("EXP2_BITS_ANT", spec, subdim=False, uops_sha=shas)
    dvo.OPS.append(op)
    dvo._SUB_OPCODE_FOR_NAME[op.name] = row
    dvo.CUSTOM_DVE_SPECS[op.name] = spec
    op.compile("v3")
    return op


def _build():
    import concourse.bacc as bacc
    import concourse.mybir as mybir
    from concourse.tile import TileContext

    EXP2 = _register_exp2_op()

    F32 = mybir.dt.float32
    F32R = mybir.dt.float32r
    BF16 = mybir.dt.bfloat16
    FP16 = mybir.dt.float16
    I16 = mybir.dt.int16
    EXP = mybir.ActivationFunctionType.Exp
    TANH = mybir.ActivationFunctionType.Tanh
    COPY = mybir.ActivationFunctionType.Copy
    MULT = mybir.AluOpType.mult

    kdebug = bool(os.environ.get("KDEBUG"))
    nc = bacc.Bacc(None, target_bir_lowering=False)
    qxT = nc.dram_tensor("qxT", [B, 2, 128, Q], BF16, kind="ExternalInput")
    kvT = nc.dram_tensor("kvT", [B, 2, 128, K], BF16, kind="ExternalInput")
    ebp = nc.dram_tensor("ebp", [NQC, 128, NKT, 512], BF16, kind="ExternalInput")
    ident = nc.dram_tensor("ident", [128, 128], BF16, kind="ExternalInput")
    expbm = nc.dram_tensor("expbm", [B, 128, NKT], F32, kind="ExternalInput")
    wq = nc.dram_tensor("wq", [2, 128, 128], BF16, kind="ExternalInput")
    wkvg = nc.dram_tensor("wkvg", [2, 128, 3 * D], BF16, kind="ExternalInput")
    wobg = nc.dram_tensor("wobg", [D, C + 1], F32R, kind="ExternalInput")
    outT = nc.dram_tensor("outT", [B, 2, 128, Q], FP16, kind="ExternalOutput")
    if kdebug:
        dbg_gp = nc.dram_tensor("dbg_gp", [32, Q], F32, kind="ExternalOutput")
        dbg_po = nc.dram_tensor("dbg_po", [D + 1, 512], F32, kind="ExternalOutput")
        dbg_wt = nc.dram_tensor("dbg_wt", [2, 128, 1024], BF16, kind="ExternalOutput")
        dbg_kq = nc.dram_tensor("dbg_kq", [128, 512 + Q], BF16, kind="ExternalOutput")
        dbg_vt = nc.dram_tensor("dbg_vt", [128, NKT * (D + 1)], BF16, kind="ExternalOutput")
        dbg_ax = nc.dram_tensor("dbg_ax", [4, 128, K], BF16, kind="ExternalOutput")
        dbg_eb = nc.dram_tensor("dbg_eb", [128, NKT, 512], BF16, kind="ExternalOutput")

    with TileContext(nc) as tc:
        with (
            tc.tile_pool(name="ld", bufs=1) as ld,
            tc.tile_pool(name="pers", bufs=1) as pers,
            tc.tile_pool(name="w0p", bufs=4) as w0pool,
            tc.tile_pool(name="wp", bufs=6) as wpool,
            tc.tile_pool(name="ep", bufs=2) as epool,
            tc.tile_pool(name="ob", bufs=3) as obpool,
            tc.tile_pool(name="ps_sc", bufs=2, space="PSUM") as ps_sc,
            tc.tile_pool(name="ps_o", bufs=2, space="PSUM") as ps_o,
            tc.tile_pool(name="ps_m", bufs=2, space="PSUM") as ps_m,
        ):
            # ---- weights first (small), then per-batch activations ----
            wkvg_sb, wq_sb = [], []
            for ch in range(2):
                t = pers.tile([128, 3 * D], BF16, name=f"wkvg_sb{ch}")
                nc.sync.dma_start(out=t[:, :], in_=wkvg[ch, :, :])
                wkvg_sb.append(t)
            wk_sb = [t[:, 0:D] for t in wkvg_sb]
            wv_sb = [t[:, D:2 * D] for t in wkvg_sb]
            wg_sb = [t[:, 2 * D:3 * D] for t in wkvg_sb]

            kv_all, qx_all = {}, {}

            def load_acts(b):
                for ch in range(2):
                    t = ld.tile([128, K], BF16, name=f"kv{b}{ch}", tag=f"kv{b}{ch}")
                    nc.sync.dma_start(out=t[:, :], in_=kvT[b, ch, :, :])
                    kv_all[(b, ch)] = t
                for ch in range(2):
                    t = ld.tile([128, Q], BF16, name=f"qx{b}{ch}", tag=f"qx{b}{ch}")
                    nc.sync.dma_start(out=t[:, :], in_=qxT[b, ch, :, :])
                    qx_all[(b, ch)] = t

            load_acts(0)
            for ch in range(2):
                t = pers.tile([128, 128], BF16, name=f"wq_sb{ch}")
                nc.sync.dma_start(out=t[:, :], in_=wq[ch, :, :])
                wq_sb.append(t)

            # small consts
            bm_sb = []
            for b in range(B):
                t = pers.tile([128, NKT], F32, name=f"bm_sb{b}")
                nc.sync.dma_start(out=t[:, :], in_=expbm[b, :, :])
                bm_sb.append(t)
            wobg_sb = pers.tile([D, C + 1], F32R, name="wobg_sb")
            nc.sync.dma_start(out=wobg_sb[:, :], in_=wobg[:, :])
            wo_sb = wobg_sb[:, 0:C]
            bg_sb = wobg_sb[:, C:C + 1].bitcast(F32)
            al_sb = pers.tile([128, 1024], F32, name="al_sb")
            nc.vector.memset(al_sb[:, :], BSHIFT + ALPHA)
            sb_sb = pers.tile([128, 1], F32, name="sb_sb")
            nc.gpsimd.memset(sb_sb[:, :], SCAL_BIAS)
            # ones row at partition 32 (matmul wants lhsT/rhs base aligned:
            # the moving l row lives at partition 32 of posb)
            ones32_sb = pers.tile([D + 1, D], F32, name="ones32_sb")
            nc.gpsimd.memset(ones32_sb[:, :], 1.0)
            id_sb = pers.tile([128, 128], BF16, name="id_sb")
            nc.sync.dma_start(out=id_sb[:, :], in_=ident[:, :])

            # exp(bias_pair) preload: whole head slice in SBUF (64KB/part)
            ebp_sb = pers.tile([128, NQC, NKT * 512], BF16, name="ebp_sb")
            ebp_loaded = set()

            def load_ebp(qc):
                if qc in ebp_loaded:
                    return
                ebp_loaded.add(qc)
                nc.sync.dma_start(out=ebp_sb[:, qc, :], in_=ebp[qc, :, :, :]
                                  .rearrange("p t q -> p (t q)"))

            load_ebp(0)
            load_ebp(1)
            load_acts(1)
            load_ebp(2)
            load_ebp(3)

            # ---- per-batch projections ----
            qT_rep, kT_sb, v_sb, gp_sb = {}, {}, {}, {}

            def emit_prologue(b):
                qx_b = [qx_all[(b, ch)] for ch in range(2)]
                kv_b = [kv_all[(b, ch)] for ch in range(2)]
                qT = pers.tile([128, Q], BF16, name=f"qT{b}")
                gp1 = pers.tile([32, Q], F32, name=f"gp1{b}")
                kT = pers.tile([128, 512], BF16, name=f"kT{b}")
                vt = pers.tile([128, NKT, D + 1], BF16, name=f"v{b}")
                qT_rep[b] = qT; kT_sb[b] = kT; v_sb[b] = vt; gp_sb[b] = gp1

                # k.T in strip layout: strip j (partitions 32j) holds tiles
                # {4g+j} at free cols g*128.  kvT is host-permuted so pos
                # 4j+g holds tile 4g+j -> moving data is contiguous.
                ps = ps_m.tile([128, 512], F32, tag="m", name=f"pk{b}")
                for ch in range(2):
                    for j in range(4):
                        nc.tensor.matmul(
                            ps[32 * j:32 * j + 32, :], wk_sb[ch][:, :],
                            kv_b[ch][:, j * 512:(j + 1) * 512],
                            start=(ch == 0), stop=(ch == 1),
                            tile_position=(0, 32 * j))
                nc.scalar.activation(kT[:, :], ps[:, :], COPY)

                # v tiles [k-partitions, d] + exp(bm) ones col; the exp(bm)
                # row scale (bias_mask fold: w and l scale alike) is applied
                # during PSUM evacuation via tensor_scalar_mul.
                for p in range(8):
                    ps = ps_m.tile([128, 2, D], F32, tag="m", name=f"pv{b}{p}",
                                   padded_shape=[128, 2, 256])
                    for i in range(2):
                        kt = 2 * p + i
                        pos = 4 * (kt % 4) + kt // 4
                        for ch in range(2):
                            nc.tensor.matmul(
                                ps[:, i, :],
                                kv_b[ch][:, pos * 128:(pos + 1) * 128],
                                wv_sb[ch][:, :], start=(ch == 0), stop=(ch == 1))
                    for i in range(2):
                        kt = 2 * p + i
                        nc.vector.tensor_scalar_mul(
                            vt[:, kt, 0:D], ps[:, i, :],
                            bm_sb[b][:, kt:kt + 1])
                # ones columns (scaled by exp(bm)) for the denominator
                nc.vector.tensor_copy(
                    vt[:, :, D:D + 1].rearrange("p k o -> p (k o)"),
                    bm_sb[b][:, :])

                # q.T replicated into 4 partition strips via 4x-duplicated
                # weight columns (host-prepared); scale folded: sD*log2e*128
                for qc in range(NQC):
                    ps = ps_m.tile([128, 512], F32, tag="m", name=f"pq{b}{qc}")
                    for ch in range(2):
                        nc.tensor.matmul(
                            ps[:, :], wq_sb[ch][:, :],
                            qx_b[ch][:, qc * 512:(qc + 1) * 512],
                            start=(ch == 0), stop=(ch == 1))
                    nc.scalar.activation(qT[:, qc * 512:(qc + 1) * 512],
                                         ps[:, :], COPY)

                # gate: tanh(z/2 + bg/2); (tanh+1)/2 with the 1/2 in Wv
                for qc in range(NQC):
                    ps = ps_m.tile([32, 512], F32, tag="m", name=f"pg{b}{qc}",
                                   padded_shape=[128, 512])
                    for ch in range(2):
                        nc.tensor.matmul(
                            ps[:, :], wg_sb[ch][:, :],
                            qx_b[ch][:, qc * 512:(qc + 1) * 512],
                            start=(ch == 0), stop=(ch == 1))
                    gt = epool.tile([32, 512], F32, tag="gt", name=f"gt{b}{qc}")
                    nc.scalar.activation(gt[:, :], ps[:, :],
                                         TANH, bias=bg_sb, scale=0.5)
                    nc.vector.tensor_scalar_add(
                        gp1[:, qc * 512:(qc + 1) * 512], gt[:, :], 1.0)

            emit_prologue(0)

            # ---- main attention loop (b outer: b1 acts can arrive late) ----
            from collections import deque
            workq = deque()

            def make_ep(qc, b, po):
                # po rows 0:32 = o (all 16 k-tiles), row 32 = l.  Evacuate
                # once; broadcast l across partitions via the ones matmul;
                # gate+normalize on [32,512].  Split in two so the PE-side
                # Wo matmuls trail the GpSimd gating chain by a tile.
                st = {}

                def ep_part1():
                    posb = epool.tile([D + 1, 512], F32R, tag="posb",
                                      name=f"posb{qc}{b}")
                    nc.scalar.activation(posb[:, :], po[0:D + 1, :], COPY)
                    if kdebug and b == 0 and qc == 0:
                        for ch in range(2):
                            nc.sync.dma_start(
                                out=dbg_ax[ch], in_=kv_all[(0, ch)][:, :])
                            nc.sync.dma_start(
                                out=dbg_ax[2 + ch], in_=qx_all[(0, ch)][:, :])
                        nc.sync.dma_start(out=dbg_eb[:, :, :],
                                          in_=ebp_sb[:, 0, :]
                                          .rearrange("p (t q) -> p t q", q=512))
                        nc.sync.dma_start(out=dbg_po[:, :],
                                          in_=posb[:, :].bitcast(F32))
                        nc.sync.dma_start(out=dbg_gp[:, :], in_=gp_sb[0][:, :])
                        nc.sync.dma_start(out=dbg_kq[:, 0:512],
                                          in_=kT_sb[0][:, :])
                        nc.sync.dma_start(out=dbg_kq[:, 512:],
                                          in_=qT_rep[0][:, :])
                        nc.sync.dma_start(
                            out=dbg_vt[:, :],
                            in_=v_sb[0][:, :, :].rearrange("p k o -> p (k o)"))
                    psl = ps_m.tile([D, 512], F32, tag="m",
                                    name=f"psl{qc}{b}",
                                    padded_shape=[128, 512])
                    nc.tensor.matmul(
                        psl[:, :], ones32_sb[D:D + 1, :].bitcast(F32R),
                        posb[D:D + 1, :], start=True, stop=True)
                    rlt = epool.tile([D, 512], F32, tag="rl",
                                     name=f"rl{qc}{b}")
                    nc.vector.reciprocal_approx_fast(rlt[:, :], psl[:, :])
                    og = epool.tile([D, 512], F32R, tag="og",
                                    name=f"og{qc}{b}")
                    nc.gpsimd.tensor_tensor(
                        og[:, :], gp_sb[b][:, qc * 512:(qc + 1) * 512],
                        posb[0:D, :], op=MULT)
                    og2 = epool.tile([D, 512], F32R, tag="og2",
                                     name=f"og2{qc}{b}")
                    nc.gpsimd.tensor_tensor(og2[:, :], og[:, :], rlt[:, :],
                                            op=MULT)
                    st["og2"] = og2

                def ep_part2():
                    og2 = st["og2"]
                    for half in range(2):
                        pp = ps_m.tile([128, 512], F32, tag="m",
                                       name=f"pp{qc}{b}{half}")
                        nc.tensor.matmul(
                            pp[:, :], wo_sb[:, half * 128:(half + 1) * 128],
                            og2[:, :], start=True, stop=True)
                        ot = obpool.tile([128, 512], FP16, tag="ot",
                                         name=f"ot{qc}{b}{half}")
                        if half == 0:
                            nc.vector.tensor_copy(ot[:, :], pp[:, :])
                        else:
                            nc.scalar.activation(ot[:, :], pp[:, :], COPY)
                        nc.sync.dma_start(
                            out=outT[b, half, :, qc * 512:(qc + 1) * 512],
                            in_=ot[:, :])

                return ep_part1, ep_part2

            for b in range(B):
                if b == 1:
                    emit_prologue(1)
                for qc in range(NQC):
                    po = ps_o.tile([128, 512], F32, tag="o", name=f"po{qc}{b}")
                    for g in range(4):
                        for X in range(2):
                            tix = 2 * g + X
                            dve = tix in DVE_TILES
                            has_id = tix in ID_TILES
                            ks = (4 * g + 2 * X) * 512
                            sc = ps_sc.tile([128, 1024], F32, tag="sc",
                                            name=f"s{qc}{b}{g}{X}")
                            if has_id:
                                # bias_pair into PSUM via identity matmuls
                                # (y-domain bias slice); scores accumulate.
                                for jj in range(2):
                                    nc.tensor.matmul(
                                        sc[:, jj * 512:(jj + 1) * 512],
                                        id_sb[:, :],
                                        ebp_sb[:, qc,
                                               ks + jj * 512:ks + (jj + 1) * 512],
                                        start=True, stop=False)
                            # score matmuls interleaved with trailing work
                            # (one queue pop each) so the PE stream stays
                            # dense -> the HAM clock-gate stays at 8/8.
                            for jj in range(2):
                                j = 2 * X + jj
                                nc.tensor.matmul(
                                    sc[:, jj * 512:(jj + 1) * 512],
                                    kT_sb[b][32 * j:32 * j + 32,
                                             g * 128:(g + 1) * 128],
                                    qT_rep[b][32 * j:32 * j + 32,
                                              qc * 512:(qc + 1) * 512],
                                    start=not has_id, stop=True,
                                    tile_position=(32 * j, 0))
                                if workq:
                                    workq.popleft()()
                            wt = wpool.tile([128, 1024], BF16, tag="w",
                                            name=f"w{qc}{b}{g}{X}")
                            if has_id:
                                if dve:
                                    nc.vector._custom_dve(
                                        EXP2, out=wt[:, :].bitcast(I16),
                                        in0=sc[:, :], in1=al_sb[:, :],
                                        s0=MAGIC, s1=EXP_A, imm2=EXP_B)
                                else:
                                    nc.scalar.activation(wt[:, :], sc[:, :],
                                                         EXP, bias=sb_sb[:, :],
                                                         scale=SCAL_SCALE)
                            else:
                                wt0 = w0pool.tile([128, 1024], BF16, tag="w0",
                                                  name=f"w0{qc}{b}{g}{X}")
                                if dve:
                                    nc.vector._custom_dve(
                                        EXP2, out=wt0[:, :].bitcast(I16),
                                        in0=sc[:, :], in1=al_sb[:, :],
                                        s0=MAGIC, s1=EXP_A, imm2=EXP_B)
                                else:
                                    nc.scalar.activation(wt0[:, :], sc[:, :],
                                                         EXP, bias=sb_sb[:, :],
                                                         scale=SCAL_SCALE)
                                nc.vector.tensor_tensor(
                                    wt[:, :], wt0[:, :],
                                    ebp_sb[:, qc, ks:ks + 1024], op=MULT)
                            if (kdebug and b == 0 and qc == 0 and tix == 1
                                    and not has_id):
                                nc.sync.dma_start(out=dbg_wt[0], in_=wt0[:, :])
                                nc.sync.dma_start(out=dbg_wt[1], in_=wt[:, :])
                            while len(workq) > 4:
                                workq.popleft()()

                            def make_oj(b, g, X, po, wt, i):
                                p = 2 * g + X

                                def emit_o():
                                    kt = 4 * g + 2 * X + i
                                    nc.tensor.matmul(
                                        po[0:D + 1, :], v_sb[b][:, kt, :],
                                        wt[:, i * 512:(i + 1) * 512],
                                        start=(p == 0 and i == 0),
                                        stop=(p == 7 and i == 1))
                                return emit_o
                            workq.append(make_oj(b, g, X, po, wt, 0))
                            workq.append(make_oj(b, g, X, po, wt, 1))
                            if tix == 7:
                                ep1, ep2 = make_ep(qc, b, po)
                                workq.append(ep1)
                                workq.append(ep2)

            while workq:
                workq.popleft()()
    nc.compile()
    return nc


def _get_nc():
    if "nc" not in _CACHE:
        _CACHE["nc"] = _build()
    return _CACHE["nc"]


def kernel(q_x, kv_x, bias_mask, bias_pair, Wq, Wk, Wv, Wo, bo, Wg, bg):
    from concourse.bass_utils import run_bass_kernel_spmd

    nc = _get_nc()
    f32 = np.float32
    q_x = np.asarray(q_x, f32); kv_x = np.asarray(kv_x, f32)
    bias_mask = np.asarray(bias_mask, f32); bias_pair = np.asarray(bias_pair, f32)
    Wq = np.asarray(Wq, f32); Wk = np.asarray(Wk, f32); Wv = np.asarray(Wv, f32)
    Wo = np.asarray(Wo, f32); bo = np.asarray(bo, f32); Wg = np.asarray(Wg, f32)
    bg = np.asarray(bg, f32)

    import ml_dtypes
    _bf16 = ml_dtypes.bfloat16
    sD = 1.0 / math.sqrt(D)
    yscale = LOG2E * 128.0
    qxT_dev = np.ascontiguousarray(
        q_x.transpose(0, 2, 1).reshape(B, 2, 128, Q)).astype(_bf16)
    # kvT with k-tiles permuted: position 4j+g holds tile 4g+j, so the
    # k.T strip matmuls read contiguous moving data.
    kv_t = kv_x.transpose(0, 2, 1).reshape(B, 2, 128, NKT, 128)
    perm = [4 * (p % 4) + p // 4 for p in range(NKT)]
    kvT_dev = np.ascontiguousarray(
        kv_t[:, :, :, perm, :].reshape(B, 2, 128, K)).astype(_bf16)
    bm_dev = np.ascontiguousarray(
        np.exp(bias_mask.reshape(B, NKT, 128).transpose(0, 2, 1)))

    def wsplit(W, h, scale=1.0):
        # [2, 128, D] view of (W_h * scale).T with W_h = W[h*D:(h+1)*D, :]
        return np.ascontiguousarray(
            (W[h * D:(h + 1) * D, :] * scale).T.reshape(2, 128, D))

    def wrep(W, h, scale=1.0):
        # weight columns duplicated 4x -> M=128 matmul emits 4 replicas
        wt = wsplit(W, h, scale)                       # [2, 128, D]
        return np.ascontiguousarray(np.tile(wt, (1, 1, 4)))

    in_maps = []
    for h in range(H):
        bp = bias_pair[0, h].T                                 # [K, Q]
        # per k-tile: ID_TILES get the y-domain bias (PE identity-matmul
        # injection), others get exp(bias) (DVE multiply into the weights)
        bpt = bp.reshape(NKT, 128, NQC, 512)
        ebp = np.empty_like(bpt)
        for kt in range(NKT):
            tix = 2 * (kt // 4) + (kt % 4) // 2
            if tix in ID_TILES:
                ebp[kt] = bpt[kt] * yscale
            else:
                ebp[kt] = np.exp(bpt[kt])
        ebp_dev = np.ascontiguousarray(
            ebp.astype(_bf16).transpose(2, 1, 0, 3))
        wkvg_h = np.concatenate(
            [wsplit(Wk, h), wsplit(Wv, h, 0.5), wsplit(Wg, h)],
            axis=2).astype(_bf16)
        wobg_h = np.concatenate(
            [np.ascontiguousarray(Wo[:, h * D:(h + 1) * D].T),
             0.5 * bg[h * D:(h + 1) * D, None]], axis=1)
        in_maps.append({
            "qxT": qxT_dev, "kvT": kvT_dev,
            "ebp": ebp_dev,
            "ident": np.eye(128, dtype=_bf16),
            "expbm": bm_dev,
            "wq": wrep(Wq, h, sD * yscale).astype(_bf16),
            "wkvg": np.ascontiguousarray(wkvg_h),
            "wobg": np.ascontiguousarray(wobg_h.astype(f32)),
        })

    try:
        res = run_bass_kernel_spmd(nc, in_maps, core_ids=list(range(H)))
    except Exception:
        # rare transient accelerator fault — one retry after a short pause
        import time as _time
        _time.sleep(5)
        res = run_bass_kernel_spmd(nc, in_maps, core_ids=list(range(H)))
    out = np.zeros((B, Q, C), f32)
    for h in range(H):
        p = res.results[h]["outT"].astype(f32).reshape(B, C, Q)
        out += p.transpose(0, 2, 1)
    out += bo
    return out


# revision 33
# speedup vs baseline: 1.1862x; 1.1535x over previous
"""Gated attention-with-pair-bias kernel for 8 Trainium2 NeuronCores.

Problem: B=2, Q=K=2048, C=256, H=8 heads, D=32 per head.
  q = (q_x @ Wq.T)/sqrt(D); k = kv_x @ Wk.T; v = kv_x @ Wv.T   (per head h)
  S = q @ k.T + bias_mask + bias_pair; w = softmax_k(S)
  o = (w @ v) * sigmoid(q_x @ Wg.T + bg); out = o @ Wo.T + bo

Sharding: one head per core; each core does both batch elements so each
head's bias slice is read from HBM exactly once.

v2 design (vs v1): the PE no longer injects bias_pair via identity
matmuls.  exp(s + bp) = exp(s) * exp(bp): the host precomputes
exp(bias_pair) in bf16 and the DVE multiplies it into the exp output
(bf16 x bf16 tensor_tensor, 2 elem/cycle/lane).  Score matmuls are now
single start/stop ops in 2 concurrent 32-row PE bands (no PSUM RMW),
and both k-tiles of a group accumulate o into the SAME po partitions
0:33 (32 o dims + the softmax-denominator ones column), so the old
97-row even/odd merge matmul is gone; only a 1-row ones matmul remains
to broadcast l across partitions for the division.
bias_mask folds into v multiplicatively (exp(bm) row scaling applied
during the PSUM->SBUF evacuation of v, ones column included).
kvT is stored with k-tiles permuted (pos 4j+g holds tile 4g+j) so the
k.T strip matmuls stream contiguous SBUF.
The gate uses tanh (same ScalarE table as Exp -> zero table reloads):
sigmoid(z) = (tanh(z/2)+1)/2, the 1/2 folded into Wv.
The division by l commutes past the output projection: og = (gate
combo) * o * (1/l) on [32,512] tiles (GpSimd), then Wo matmul, copy out.
A knob (DVE_EXP_TILES) can move some exp tiles from ScalarE to a custom
DVE op (EXP2_BITS_ANT) that builds the bf16 BIT PATTERN of 2^(y/128)
arithmetically: round/frac split via the +1.5*2^30 magic constant, a
deg-2 mantissa polynomial, and an int16 store whose bits are the bf16
weight.  Both paths share one uniform scale 2^c that cancels in o/l.
"""

import math
import os
import sys

sys.path.insert(0, "/opt/trn_rl_repo")

import numpy as np

H, D, B, Q, K, C = 8, 32, 2, 2048, 2048, 256
NQC = 4          # q chunks of 512
NKT = K // 128   # 16 k tiles

# exp2-bit-construction constants (fit offline)
MAGIC = 1.5 * 2 ** 30
EXP_A = 0.00255          # t^2 coeff (2^7-scaled domain)
EXP_B = 0.996            # t coeff
ALPHA = 53.7             # mantissa-poly constant, added post-round
BSHIFT = 16256.0         # 127*128: bf16 exponent bias in 2^7 units
C_CENTER = 0.5033798133168   # log2(w) - y/128 (uniform; cancels in o/l)
LOG2E = math.log2(math.e)
SCAL_SCALE = math.log(2.0) / 128.0
SCAL_BIAS = C_CENTER * math.log(2.0)

# per-(qc,b) psum-tile indices (2g+X) whose exp runs on the DVE custom op
# instead of ScalarE.  Tune for engine balance.
_dve_env = os.environ.get("DVE_EXP_TILES", "")
DVE_TILES = frozenset(int(x) for x in _dve_env.split(",") if x != "")
# tile indices whose bias_pair is injected on the PE via identity matmuls
# (PSUM pre-load, v1-style) instead of the DVE exp(bp) multiply.  More id
# tiles -> more PE work but denser PE fill (keeps the HAM clock-gate at
# 8/8) and less DVE work.
_id_env = os.environ.get("ID_TILES", "0,3,6")
ID_TILES = frozenset(int(x) for x in _id_env.split(",") if x != "")

_CACHE = {}


def _register_exp2_op():
    """Register the custom DVE op (in-process; documented extension API)."""
    import concourse.dve_ops as dvo
    from concourse.dve_spec import Spec, Src0, Src1, C0, C1, C2, lower
    from concourse.dve_uop import DveOpSpec

    if "EXP2_BITS_ANT" in dvo._SUB_OPCODE_FOR_NAME:
        return next(o for o in dvo.OPS if o.name == "EXP2_BITS_ANT")

    m = Src0 + C0
    r = m - C0
    t = Src0 - r
    h3 = ((t * C1) + C2) * t
    body = (h3 + r) + Src1

    def ref(in0, in1, s0, s1, imm2):
        f32 = np.float32
        mm = (in0.astype(f32) + f32(s0)).astype(f32)
        rr = (mm - f32(s0)).astype(f32)
        tt = (in0.astype(f32) - rr).astype(f32)
        hh = (((tt * f32(s1)).astype(f32) + f32(imm2)).astype(f32) * tt).astype(f32)
        return ((hh + rr).astype(f32) + in1.astype(f32)).astype(f32)

    spec = Spec(body=body, reference=ref)
    row = dvo._CUSTOM_DVE_ROW_BASE + len(dvo.OPS)
    assert row < 0x20
    shas = {v: DveOpSpec(name="EXP2_BITS_ANT", opcode=row,
                         uops=lower(spec, ver=v), rd1_en=True).sha(v)
            for v in ("v3", "v4")}
    op = dvo.DveOp("EXP2_BITS_ANT", spec, subdim=False, uops_sha=shas)
    dvo.OPS.append(op)
    dvo._SUB_OPCODE_FOR_NAME[op.name] = row
    dvo.CUSTOM_DVE_SPECS[op.name] = spec
    op.compile("v3")
    return op


def _build():
    import concourse.bacc as bacc
    import concourse.mybir as mybir
    from concourse.tile import TileContext

    EXP2 = _register_exp2_op()

    F32 = mybir.dt.float32
    F32R = mybir.dt.float32r
    BF16 = mybir.dt.bfloat16
    FP16 = mybir.dt.float16
    I16 = mybir.dt.int16
    EXP = mybir.ActivationFunctionType.Exp
    TANH = mybir.ActivationFunctionType.Tanh
    COPY = mybir.ActivationFunctionType.Copy
    MULT = mybir.AluOpType.mult

    kdebug = bool(os.environ.get("KDEBUG"))
    nc = bacc.Bacc(None, target_bir_lowering=False)
    qxT = nc.dram_tensor("qxT", [B, 2, 128, Q], BF16, kind="ExternalInput")
    kvT = nc.dram_tensor("kvT", [B, 2, 128, K], BF16, kind="ExternalInput")
    ebp = nc.dram_tensor("ebp", [NQC, 128, NKT, 512], BF16, kind="ExternalInput")
    ident = nc.dram_tensor("ident", [128, 128], BF16, kind="ExternalInput")
    expbm = nc.dram_tensor("expbm", [B, 128, NKT], F32, kind="ExternalInput")
    wq = nc.dram_tensor("wq", [2, 128, 128], BF16, kind="ExternalInput")
    wkvg = nc.dram_tensor("wkvg", [2, 128, 3 * D], BF16, kind="ExternalInput")
    wobg = nc.dram_tensor("wobg", [D, C + 1], F32R, kind="ExternalInput")
    outT = nc.dram_tensor("outT", [B, 2, 128, Q], FP16, kind="ExternalOutput")
    if kdebug:
        dbg_gp = nc.dram_tensor("dbg_gp", [32, Q], F32, kind="ExternalOutput")
        dbg_po = nc.dram_tensor("dbg_po", [D + 1, 512], F32, kind="ExternalOutput")
        dbg_wt = nc.dram_tensor("dbg_wt", [2, 128, 1024], BF16, kind="ExternalOutput")
        dbg_kq = nc.dram_tensor("dbg_kq", [128, 512 + Q], BF16, kind="ExternalOutput")
        dbg_vt = nc.dram_tensor("dbg_vt", [128, NKT * (D + 1)], BF16, kind="ExternalOutput")
        dbg_ax = nc.dram_tensor("dbg_ax", [4, 128, K], BF16, kind="ExternalOutput")
        dbg_eb = nc.dram_tensor("dbg_eb", [128, NKT, 512], BF16, kind="ExternalOutput")

    with TileContext(nc) as tc:
        with (
            tc.tile_pool(name="ld", bufs=1) as ld,
            tc.tile_pool(name="pers", bufs=1) as pers,
            tc.tile_pool(name="w0p", bufs=4) as w0pool,
            tc.tile_pool(name="wp", bufs=6) as wpool,
            tc.tile_pool(name="ep", bufs=2) as epool,
            tc.tile_pool(name="ob", bufs=3) as obpool,
            tc.tile_pool(name="ps_sc", bufs=2, space="PSUM") as ps_sc,
            tc.tile_pool(name="ps_o", bufs=2, space="PSUM") as ps_o,
            tc.tile_pool(name="ps_m", bufs=2, space="PSUM") as ps_m,
        ):
            # ---- weights first (small), then per-batch activations ----
            wkvg_sb, wq_sb = [], []
            for ch in range(2):
                t = pers.tile([128, 3 * D], BF16, name=f"wkvg_sb{ch}")
                nc.sync.dma_start(out=t[:, :], in_=wkvg[ch, :, :])
                wkvg_sb.append(t)
            wk_sb = [t[:, 0:D] for t in wkvg_sb]
            wv_sb = [t[:, D:2 * D] for t in wkvg_sb]
            wg_sb = [t[:, 2 * D:3 * D] for t in wkvg_sb]

            kv_all, qx_all = {}, {}

            def load_acts(b):
                for ch in range(2):
                    t = ld.tile([128, K], BF16, name=f"kv{b}{ch}", tag=f"kv{b}{ch}")
                    nc.sync.dma_start(out=t[:, :], in_=kvT[b, ch, :, :])
                    kv_all[(b, ch)] = t
                for ch in range(2):
                    t = ld.tile([128, Q], BF16, name=f"qx{b}{ch}", tag=f"qx{b}{ch}")
                    nc.sync.dma_start(out=t[:, :], in_=qxT[b, ch, :, :])
                    qx_all[(b, ch)] = t

            load_acts(0)
            for ch in range(2):
                t = pers.tile([128, 128], BF16, name=f"wq_sb{ch}")
                nc.sync.dma_start(out=t[:, :], in_=wq[ch, :, :])
                wq_sb.append(t)

            # small consts
            bm_sb = []
            for b in range(B):
                t = pers.tile([128, NKT], F32, name=f"bm_sb{b}")
                nc.sync.dma_start(out=t[:, :], in_=expbm[b, :, :])
                bm_sb.append(t)
            wobg_sb = pers.tile([D, C + 1], F32R, name="wobg_sb")
            nc.sync.dma_start(out=wobg_sb[:, :], in_=wobg[:, :])
            wo_sb = wobg_sb[:, 0:C]
            bg_sb = wobg_sb[:, C:C + 1].bitcast(F32)
            al_sb = pers.tile([128, 1024], F32, name="al_sb")
            nc.vector.memset(al_sb[:, :], BSHIFT + ALPHA)
            sb_sb = pers.tile([128, 1], F32, name="sb_sb")
            nc.gpsimd.memset(sb_sb[:, :], SCAL_BIAS)
            # ones row at partition 32 (matmul wants lhsT/rhs base aligned:
            # the moving l row lives at partition 32 of posb)
            ones32_sb = pers.tile([D + 1, D], F32, name="ones32_sb")
            nc.gpsimd.memset(ones32_sb[:, :], 1.0)
            id_sb = pers.tile([128, 128], BF16, name="id_sb")
            nc.sync.dma_start(out=id_sb[:, :], in_=ident[:, :])

            # exp(bias_pair) preload: whole head slice in SBUF (64KB/part)
            ebp_sb = pers.tile([128, NQC, NKT * 512], BF16, name="ebp_sb")
            ebp_loaded = set()

            def load_ebp(qc):
                if qc in ebp_loaded:
                    return
                ebp_loaded.add(qc)
                nc.sync.dma_start(out=ebp_sb[:, qc, :], in_=ebp[qc, :, :, :]
                                  .rearrange("p t q -> p (t q)"))

            load_ebp(0)
            load_ebp(1)
            load_acts(1)
            load_ebp(2)
            load_ebp(3)

            # ---- per-batch projections ----
            qT_rep, kT_sb, v_sb, gp_sb = {}, {}, {}, {}

            def emit_prologue(b):
                qx_b = [qx_all[(b, ch)] for ch in range(2)]
                kv_b = [kv_all[(b, ch)] for ch in range(2)]
                qT = pers.tile([128, Q], BF16, name=f"qT{b}")
                gp1 = pers.tile([32, Q], F32, name=f"gp1{b}")
                kT = pers.tile([128, 512], BF16, name=f"kT{b}")
                vt = pers.tile([128, NKT, D + 1], BF16, name=f"v{b}")
                qT_rep[b] = qT; kT_sb[b] = kT; v_sb[b] = vt; gp_sb[b] = gp1

                # k.T in strip layout: strip j (partitions 32j) holds tiles
                # {4g+j} at free cols g*128.  kvT is host-permuted so pos
                # 4j+g holds tile 4g+j -> moving data is contiguous.
                ps = ps_m.tile([128, 512], F32, tag="m", name=f"pk{b}")
                for ch in range(2):
                    for j in range(4):
                        nc.tensor.matmul(
                            ps[32 * j:32 * j + 32, :], wk_sb[ch][:, :],
                            kv_b[ch][:, j * 512:(j + 1) * 512],
                            start=(ch == 0), stop=(ch == 1),
                            tile_position=(0, 32 * j))
                nc.vector.tensor_copy(kT[:, :], ps[:, :])

                # v tiles [k-partitions, d] + exp(bm) ones col; the exp(bm)
                # row scale (bias_mask fold: w and l scale alike) is applied
                # during PSUM evacuation via tensor_scalar_mul.
                for p in range(8):
                    ps = ps_m.tile([128, 2, D], F32, tag="m", name=f"pv{b}{p}",
                                   padded_shape=[128, 2, 256])
                    for i in range(2):
                        kt = 2 * p + i
                        pos = 4 * (kt % 4) + kt // 4
                        for ch in range(2):
                            nc.tensor.matmul(
                                ps[:, i, :],
                                kv_b[ch][:, pos * 128:(pos + 1) * 128],
                                wv_sb[ch][:, :], start=(ch == 0), stop=(ch == 1))
                    for i in range(2):
                        kt = 2 * p + i
                        nc.vector.tensor_scalar_mul(
                            vt[:, kt, 0:D], ps[:, i, :],
                            bm_sb[b][:, kt:kt + 1])
                # ones columns (scaled by exp(bm)) for the denominator
                nc.vector.tensor_copy(
                    vt[:, :, D:D + 1].rearrange("p k o -> p (k o)"),
                    bm_sb[b][:, :])

                # q.T replicated into 4 partition strips via 4x-duplicated
                # weight columns (host-prepared); scale folded: sD*log2e*128
                for qc in range(NQC):
                    ps = ps_m.tile([128, 512], F32, tag="m", name=f"pq{b}{qc}")
                    for ch in range(2):
                        nc.tensor.matmul(
                            ps[:, :], wq_sb[ch][:, :],
                            qx_b[ch][:, qc * 512:(qc + 1) * 512],
                            start=(ch == 0), stop=(ch == 1))
                    nc.scalar.activation(qT[:, qc * 512:(qc + 1) * 512],
                                         ps[:, :], COPY)

                # gate: tanh(z/2 + bg/2); (tanh+1)/2 with the 1/2 in Wv
                for qc in range(NQC):
                    ps = ps_m.tile([32, 512], F32, tag="m", name=f"pg{b}{qc}",
                                   padded_shape=[128, 512])
                    for ch in range(2):
                        nc.tensor.matmul(
                            ps[:, :], wg_sb[ch][:, :],
                            qx_b[ch][:, qc * 512:(qc + 1) * 512],
                            start=(ch == 0), stop=(ch == 1))
                    gt = epool.tile([32, 512], F32, tag="gt", name=f"gt{b}{qc}")
                    nc.scalar.activation(gt[:, :], ps[:, :],
                                         TANH, bias=bg_sb, scale=0.5)
                    nc.vector.tensor_scalar_add(
                        gp1[:, qc * 512:(qc + 1) * 512], gt[:, :], 1.0)

            emit_prologue(0)

            # ---- main attention loop (b outer: b1 acts can arrive late) ----
            from collections import deque
            workq = deque()
            pend_ep2 = [None]

            def make_ep(qc, b, po):
                # po rows 0:32 = o (all 16 k-tiles), row 32 = l.  The gate
                # multiply reads po straight from PSUM on the DVE; only the
                # 1-row l needs an SBUF hop for the PE broadcast matmul.
                # Split in two so the PE-side Wo matmuls trail the division
                # chain by a couple of tiles.
                st = {}

                def ep_part1():
                    lrow = epool.tile([D + 1, 512], F32R, tag="lrow",
                                      name=f"lrow{qc}{b}")
                    nc.vector.tensor_copy(lrow[D:D + 1, :], po[D:D + 1, :])
                    og = epool.tile([D, 512], F32R, tag="og",
                                    name=f"og{qc}{b}")
                    nc.vector.tensor_tensor(
                        og[:, :], po[0:D, :],
                        gp_sb[b][:, qc * 512:(qc + 1) * 512], op=MULT)
                    psl = ps_m.tile([D, 512], F32, tag="m",
                                    name=f"psl{qc}{b}",
                                    padded_shape=[128, 512])
                    nc.tensor.matmul(
                        psl[:, :], ones32_sb[D:D + 1, :].bitcast(F32R),
                        lrow[D:D + 1, :], start=True, stop=True)
                    rlt = epool.tile([D, 512], F32, tag="rl",
                                     name=f"rl{qc}{b}")
                    nc.vector.reciprocal_approx_fast(rlt[:, :], psl[:, :])
                    if kdebug and b == 0 and qc == 0:
                        for ch in range(2):
                            nc.sync.dma_start(
                                out=dbg_ax[ch], in_=kv_all[(0, ch)][:, :])
                            nc.sync.dma_start(
                                out=dbg_ax[2 + ch], in_=qx_all[(0, ch)][:, :])
                        nc.sync.dma_start(out=dbg_eb[:, :, :],
                                          in_=ebp_sb[:, 0, :]
                                          .rearrange("p (t q) -> p t q", q=512))
                        nc.sync.dma_start(out=dbg_po[0:D, :],
                                          in_=og[:, :].bitcast(F32))
                        nc.sync.dma_start(out=dbg_gp[:, :], in_=gp_sb[0][:, :])
                        nc.sync.dma_start(out=dbg_kq[:, 0:512],
                                          in_=kT_sb[0][:, :])
                        nc.sync.dma_start(out=dbg_kq[:, 512:],
                                          in_=qT_rep[0][:, :])
                        nc.sync.dma_start(
                            out=dbg_vt[:, :],
                            in_=v_sb[0][:, :, :].rearrange("p k o -> p (k o)"))
                    st["og"] = og
                    st["rlt"] = rlt

                def ep_part2():
                    og2 = epool.tile([D, 512], F32R, tag="og2",
                                     name=f"og2{qc}{b}")
                    nc.vector.tensor_tensor(og2[:, :], st["og"][:, :],
                                            st["rlt"][:, :], op=MULT)
                    for half in range(2):
                        pp = ps_m.tile([128, 512], F32, tag="m",
                                       name=f"pp{qc}{b}{half}")
                        nc.tensor.matmul(
                            pp[:, :], wo_sb[:, half * 128:(half + 1) * 128],
                            og2[:, :], start=True, stop=True)
                        ot = obpool.tile([128, 512], FP16, tag="ot",
                                         name=f"ot{qc}{b}{half}")
                        if half == 0:
                            nc.vector.tensor_copy(ot[:, :], pp[:, :])
                        else:
                            nc.scalar.activation(ot[:, :], pp[:, :], COPY)
                        nc.sync.dma_start(
                            out=outT[b, half, :, qc * 512:(qc + 1) * 512],
                            in_=ot[:, :])

                return ep_part1, ep_part2

            for b in range(B):
                if b == 1:
                    emit_prologue(1)
                for qc in range(NQC):
                    po = ps_o.tile([128, 512], F32, tag="o", name=f"po{qc}{b}")
                    for g in range(4):
                        for X in range(2):
                            tix = 2 * g + X
                            dve = tix in DVE_TILES
                            has_id = tix in ID_TILES
                            ks = (4 * g + 2 * X) * 512
                            sc = ps_sc.tile([128, 1024], F32, tag="sc",
                                            name=f"s{qc}{b}{g}{X}")
                            if has_id:
                                # bias_pair into PSUM via identity matmuls
                                # (y-domain bias slice); scores accumulate.
                                for jj in range(2):
                                    nc.tensor.matmul(
                                        sc[:, jj * 512:(jj + 1) * 512],
                                        id_sb[:, :],
                                        ebp_sb[:, qc,
                                               ks + jj * 512:ks + (jj + 1) * 512],
                                        start=True, stop=False)
                            # score matmuls interleaved with trailing work
                            # (one queue pop each) so the PE stream stays
                            # dense -> the HAM clock-gate stays at 8/8.
                            for jj in range(2):
                                j = 2 * X + jj
                                nc.tensor.matmul(
                                    sc[:, jj * 512:(jj + 1) * 512],
                                    kT_sb[b][32 * j:32 * j + 32,
                                             g * 128:(g + 1) * 128],
                                    qT_rep[b][32 * j:32 * j + 32,
                                              qc * 512:(qc + 1) * 512],
                                    start=not has_id, stop=True,
                                    tile_position=(32 * j, 0))
                                if workq:
                                    workq.popleft()()
                            wt = wpool.tile([128, 1024], BF16, tag="w",
                                            name=f"w{qc}{b}{g}{X}")
                            if has_id:
                                if dve:
                                    nc.vector._custom_dve(
                                        EXP2, out=wt[:, :].bitcast(I16),
                                        in0=sc[:, :], in1=al_sb[:, :],
                                        s0=MAGIC, s1=EXP_A, imm2=EXP_B)
                                else:
                                    nc.scalar.activation(wt[:, :], sc[:, :],
                                                         EXP, bias=sb_sb[:, :],
                                                         scale=SCAL_SCALE)
                            else:
                                wt0 = w0pool.tile([128, 1024], BF16, tag="w0",
                                                  name=f"w0{qc}{b}{g}{X}")
                                if dve:
                                    nc.vector._custom_dve(
                                        EXP2, out=wt0[:, :].bitcast(I16),
                                        in0=sc[:, :], in1=al_sb[:, :],
                                        s0=MAGIC, s1=EXP_A, imm2=EXP_B)
                                else:
                                    nc.scalar.activation(wt0[:, :], sc[:, :],
                                                         EXP, bias=sb_sb[:, :],
                                                         scale=SCAL_SCALE)
                                nc.vector.tensor_tensor(
                                    wt[:, :], wt0[:, :],
                                    ebp_sb[:, qc, ks:ks + 1024], op=MULT)
                            if (kdebug and b == 0 and qc == 0 and tix == 1
                                    and not has_id):
                                nc.sync.dma_start(out=dbg_wt[0], in_=wt0[:, :])
                                nc.sync.dma_start(out=dbg_wt[1], in_=wt[:, :])
                            while len(workq) > 4:
                                workq.popleft()()

                            def make_oj(b, g, X, po, wt, i):
                                p = 2 * g + X

                                def emit_o():
                                    kt = 4 * g + 2 * X + i
                                    nc.tensor.matmul(
                                        po[0:D + 1, :], v_sb[b][:, kt, :],
                                        wt[:, i * 512:(i + 1) * 512],
                                        start=(p == 0 and i == 0),
                                        stop=(p == 7 and i == 1))
                                return emit_o
                            workq.append(make_oj(b, g, X, po, wt, 0))
                            workq.append(make_oj(b, g, X, po, wt, 1))
                            if tix == 2 and pend_ep2[0] is not None:
                                workq.append(pend_ep2[0])
                                pend_ep2[0] = None
                            if tix == 7:
                                ep1, ep2 = make_ep(qc, b, po)
                                workq.append(ep1)
                                pend_ep2[0] = ep2

            if pend_ep2[0] is not None:
                workq.append(pend_ep2[0])
                pend_ep2[0] = None
            while workq:
                workq.popleft()()
    nc.compile()
    return nc


def _get_nc():
    if "nc" not in _CACHE:
        _CACHE["nc"] = _build()
    return _CACHE["nc"]


def kernel(q_x, kv_x, bias_mask, bias_pair, Wq, Wk, Wv, Wo, bo, Wg, bg):
    from concourse.bass_utils import run_bass_kernel_spmd

    nc = _get_nc()
    f32 = np.float32
    q_x = np.asarray(q_x, f32); kv_x = np.asarray(kv_x, f32)
    bias_mask = np.asarray(bias_mask, f32); bias_pair = np.asarray(bias_pair, f32)
    Wq = np.asarray(Wq, f32); Wk = np.asarray(Wk, f32); Wv = np.asarray(Wv, f32)
    Wo = np.asarray(Wo, f32); bo = np.asarray(bo, f32); Wg = np.asarray(Wg, f32)
    bg = np.asarray(bg, f32)

    import ml_dtypes
    _bf16 = ml_dtypes.bfloat16
    sD = 1.0 / math.sqrt(D)
    yscale = LOG2E * 128.0
    qxT_dev = np.ascontiguousarray(
        q_x.transpose(0, 2, 1).reshape(B, 2, 128, Q)).astype(_bf16)
    # kvT with k-tiles permuted: position 4j+g holds tile 4g+j, so the
    # k.T strip matmuls read contiguous moving data.
    kv_t = kv_x.transpose(0, 2, 1).reshape(B, 2, 128, NKT, 128)
    perm = [4 * (p % 4) + p // 4 for p in range(NKT)]
    kvT_dev = np.ascontiguousarray(
        kv_t[:, :, :, perm, :].reshape(B, 2, 128, K)).astype(_bf16)
    bm_dev = np.ascontiguousarray(
        np.exp(bias_mask.reshape(B, NKT, 128).transpose(0, 2, 1)))

    def wsplit(W, h, scale=1.0):
        # [2, 128, D] view of (W_h * scale).T with W_h = W[h*D:(h+1)*D, :]
        return np.ascontiguousarray(
            (W[h * D:(h + 1) * D, :] * scale).T.reshape(2, 128, D))

    def wrep(W, h, scale=1.0):
        # weight columns duplicated 4x -> M=128 matmul emits 4 replicas
        wt = wsplit(W, h, scale)                       # [2, 128, D]
        return np.ascontiguousarray(np.tile(wt, (1, 1, 4)))

    in_maps = []
    for h in range(H):
        bp = bias_pair[0, h].T                                 # [K, Q]
        # per k-tile: ID_TILES get the y-domain bias (PE identity-matmul
        # injection), others get exp(bias) (DVE multiply into the weights)
        bpt = bp.reshape(NKT, 128, NQC, 512)
        ebp = np.empty_like(bpt)
        for kt in range(NKT):
            tix = 2 * (kt // 4) + (kt % 4) // 2
            if tix in ID_TILES:
                ebp[kt] = bpt[kt] * yscale
            else:
                ebp[kt] = np.exp(bpt[kt])
        ebp_dev = np.ascontiguousarray(
            ebp.astype(_bf16).transpose(2, 1, 0, 3))
        wkvg_h = np.concatenate(
            [wsplit(Wk, h), wsplit(Wv, h, 0.5), wsplit(Wg, h)],
            axis=2).astype(_bf16)
        wobg_h = np.concatenate(
            [np.ascontiguousarray(Wo[:, h * D:(h + 1) * D].T),
             0.5 * bg[h * D:(h + 1) * D, None]], axis=1)
        in_maps.append({
            "qxT": qxT_dev, "kvT": kvT_dev,
            "ebp": ebp_dev,
            "ident": np.eye(128, dtype=_bf16),
            "expbm": bm_dev,
            "wq": wrep(Wq, h, sD * yscale).astype(_bf16),
            "wkvg": np.ascontiguousarray(wkvg_h),
            "wobg": np.ascontiguousarray(wobg_h.astype(f32)),
        })

    try:
        res = run_bass_kernel_spmd(nc, in_maps, core_ids=list(range(H)))
    except Exception:
        # rare transient accelerator fault — one retry after a short pause
        import time as _time
        _time.sleep(5)
        res = run_bass_kernel_spmd(nc, in_maps, core_ids=list(range(H)))
    out = np.zeros((B, Q, C), f32)
    for h in range(H):
        p = res.results[h]["outT"].astype(f32).reshape(B, C, Q)
        out += p.transpose(0, 2, 1)
    out += bo
    return out


# revision 42
# speedup vs baseline: 1.2679x; 1.0688x over previous
"""Gated attention-with-pair-bias kernel for 8 Trainium2 NeuronCores.

Problem: B=2, Q=K=2048, C=256, H=8 heads, D=32 per head.
  q = (q_x @ Wq.T)/sqrt(D); k = kv_x @ Wk.T; v = kv_x @ Wv.T   (per head h)
  S = q @ k.T + bias_mask + bias_pair; w = softmax_k(S)
  o = (w @ v) * sigmoid(q_x @ Wg.T + bg); out = o @ Wo.T + bo

Sharding: one head per core; each core does both batch elements so each
head's bias slice is read from HBM exactly once.

v2 design (vs v1): the PE no longer injects bias_pair via identity
matmuls.  exp(s + bp) = exp(s) * exp(bp): the host precomputes
exp(bias_pair) in bf16 and the DVE multiplies it into the exp output
(bf16 x bf16 tensor_tensor, 2 elem/cycle/lane).  Score matmuls are now
single start/stop ops in 2 concurrent 32-row PE bands (no PSUM RMW),
and both k-tiles of a group accumulate o into the SAME po partitions
0:33 (32 o dims + the softmax-denominator ones column), so the old
97-row even/odd merge matmul is gone; only a 1-row ones matmul remains
to broadcast l across partitions for the division.
bias_mask folds into v multiplicatively (exp(bm) row scaling applied
during the PSUM->SBUF evacuation of v, ones column included).
kvT is stored with k-tiles permuted (pos 4j+g holds tile 4g+j) so the
k.T strip matmuls stream contiguous SBUF.
The gate uses tanh (same ScalarE table as Exp -> zero table reloads):
sigmoid(z) = (tanh(z/2)+1)/2, the 1/2 folded into Wv.
The division by l commutes past the output projection: og = (gate
combo) * o * (1/l) on [32,512] tiles (GpSimd), then Wo matmul, copy out.
A knob (DVE_EXP_TILES) can move some exp tiles from ScalarE to a custom
DVE op (EXP2_BITS_ANT) that builds the bf16 BIT PATTERN of 2^(y/128)
arithmetically: round/frac split via the +1.5*2^30 magic constant, a
deg-2 mantissa polynomial, and an int16 store whose bits are the bf16
weight.  Both paths share one uniform scale 2^c that cancels in o/l.
"""

import math
import os
import sys

sys.path.insert(0, "/opt/trn_rl_repo")

import numpy as np

H, D, B, Q, K, C = 8, 32, 2, 2048, 2048, 256
NQC = 4          # q chunks of 512
NKT = K // 128   # 16 k tiles

# exp2-bit-construction constants (fit offline)
MAGIC = 1.5 * 2 ** 30
EXP_A = 0.00255          # t^2 coeff (2^7-scaled domain)
EXP_B = 0.996            # t coeff
ALPHA = 53.7             # mantissa-poly constant, added post-round
BSHIFT = 16256.0         # 127*128: bf16 exponent bias in 2^7 units
C_CENTER = 0.5033798133168   # log2(w) - y/128 (uniform; cancels in o/l)
LOG2E = math.log2(math.e)
SCAL_SCALE = math.log(2.0) / 128.0
SCAL_BIAS = C_CENTER * math.log(2.0)

# per-(qc,b) psum-tile indices (2g+X) whose exp runs on the DVE custom op
# instead of ScalarE.  Tune for engine balance.
_dve_env = os.environ.get("DVE_EXP_TILES", "")
DVE_TILES = frozenset(int(x) for x in _dve_env.split(",") if x != "")
# tile indices whose bias_pair is injected on the PE via identity matmuls
# (PSUM pre-load, v1-style) instead of the DVE exp(bp) multiply.  More id
# tiles -> more PE work but denser PE fill (keeps the HAM clock-gate at
# 8/8) and less DVE work.
_id_env = os.environ.get("ID_TILES", "0,3,6")
ID_TILES = frozenset(int(x) for x in _id_env.split(",") if x != "")

_CACHE = {}


def _register_exp2_op():
    """Register the custom DVE op (in-process; documented extension API)."""
    import concourse.dve_ops as dvo
    from concourse.dve_spec import Spec, Src0, Src1, C0, C1, C2, lower
    from concourse.dve_uop import DveOpSpec

    if "EXP2_BITS_ANT" in dvo._SUB_OPCODE_FOR_NAME:
        return next(o for o in dvo.OPS if o.name == "EXP2_BITS_ANT")

    m = Src0 + C0
    r = m - C0
    t = Src0 - r
    h3 = ((t * C1) + C2) * t
    body = (h3 + r) + Src1

    def ref(in0, in1, s0, s1, imm2):
        f32 = np.float32
        mm = (in0.astype(f32) + f32(s0)).astype(f32)
        rr = (mm - f32(s0)).astype(f32)
        tt = (in0.astype(f32) - rr).astype(f32)
        hh = (((tt * f32(s1)).astype(f32) + f32(imm2)).astype(f32) * tt).astype(f32)
        return ((hh + rr).astype(f32) + in1.astype(f32)).astype(f32)

    spec = Spec(body=body, reference=ref)
    row = dvo._CUSTOM_DVE_ROW_BASE + len(dvo.OPS)
    assert row < 0x20
    shas = {v: DveOpSpec(name="EXP2_BITS_ANT", opcode=row,
                         uops=lower(spec, ver=v), rd1_en=True).sha(v)
            for v in ("v3", "v4")}
    op = dvo.DveOp("EXP2_BITS_ANT", spec, subdim=False, uops_sha=shas)
    dvo.OPS.append(op)
    dvo._SUB_OPCODE_FOR_NAME[op.name] = row
    dvo.CUSTOM_DVE_SPECS[op.name] = spec
    op.compile("v3")
    return op


def _build():
    import concourse.bacc as bacc
    import concourse.mybir as mybir
    from concourse.tile import TileContext

    EXP2 = _register_exp2_op()

    F32 = mybir.dt.float32
    F32R = mybir.dt.float32r
    BF16 = mybir.dt.bfloat16
    FP16 = mybir.dt.float16
    I16 = mybir.dt.int16
    EXP = mybir.ActivationFunctionType.Exp
    TANH = mybir.ActivationFunctionType.Tanh
    COPY = mybir.ActivationFunctionType.Copy
    MULT = mybir.AluOpType.mult

    kdebug = bool(os.environ.get("KDEBUG"))
    nc = bacc.Bacc(None, target_bir_lowering=False)
    qxT = nc.dram_tensor("qxT", [B, 2, 128, Q], BF16, kind="ExternalInput")
    kvT = nc.dram_tensor("kvT", [B, 2, 128, K], BF16, kind="ExternalInput")
    ebp = nc.dram_tensor("ebp", [NQC, 128, NKT, 512], BF16, kind="ExternalInput")
    ident = nc.dram_tensor("ident", [128, 128], BF16, kind="ExternalInput")
    expbm = nc.dram_tensor("expbm", [B, 128, NKT], F32, kind="ExternalInput")
    wq = nc.dram_tensor("wq", [2, 128, 128], BF16, kind="ExternalInput")
    wkvg = nc.dram_tensor("wkvg", [2, 128, 3 * D], BF16, kind="ExternalInput")
    wobg = nc.dram_tensor("wobg", [D, C + 1], F32R, kind="ExternalInput")
    outT = nc.dram_tensor("outT", [B, 2, 128, Q], FP16, kind="ExternalOutput")
    if kdebug:
        dbg_gp = nc.dram_tensor("dbg_gp", [32, Q], F32, kind="ExternalOutput")
        dbg_po = nc.dram_tensor("dbg_po", [D + 1, 512], F32, kind="ExternalOutput")
        dbg_wt = nc.dram_tensor("dbg_wt", [2, 128, 1024], BF16, kind="ExternalOutput")
        dbg_kq = nc.dram_tensor("dbg_kq", [128, 512 + Q], BF16, kind="ExternalOutput")
        dbg_vt = nc.dram_tensor("dbg_vt", [128, NKT * (D + 1)], BF16, kind="ExternalOutput")
        dbg_ax = nc.dram_tensor("dbg_ax", [4, 128, K], BF16, kind="ExternalOutput")
        dbg_eb = nc.dram_tensor("dbg_eb", [128, NKT, 512], BF16, kind="ExternalOutput")

    with TileContext(nc) as tc:
        with (
            tc.tile_pool(name="ld", bufs=1) as ld,
            tc.tile_pool(name="pers", bufs=1) as pers,
            tc.tile_pool(name="w0p", bufs=4) as w0pool,
            tc.tile_pool(name="wp", bufs=6) as wpool,
            tc.tile_pool(name="ep", bufs=2) as epool,
            tc.tile_pool(name="ob", bufs=3) as obpool,
            tc.tile_pool(name="ps_sc", bufs=2, space="PSUM") as ps_sc,
            tc.tile_pool(name="ps_o", bufs=2, space="PSUM") as ps_o,
            tc.tile_pool(name="ps_m", bufs=2, space="PSUM") as ps_m,
        ):
            # ---- weights first (small), then per-batch activations ----
            wkvg_sb, wq_sb = [], []
            for ch in range(2):
                t = pers.tile([128, 3 * D], BF16, name=f"wkvg_sb{ch}")
                nc.sync.dma_start(out=t[:, :], in_=wkvg[ch, :, :])
                wkvg_sb.append(t)
            wk_sb = [t[:, 0:D] for t in wkvg_sb]
            wv_sb = [t[:, D:2 * D] for t in wkvg_sb]
            wg_sb = [t[:, 2 * D:3 * D] for t in wkvg_sb]

            kv_all, qx_all = {}, {}

            def load_acts(b):
                for ch in range(2):
                    t = ld.tile([128, K], BF16, name=f"kv{b}{ch}", tag=f"kv{b}{ch}")
                    nc.sync.dma_start(out=t[:, :], in_=kvT[b, ch, :, :])
                    kv_all[(b, ch)] = t
                for ch in range(2):
                    t = ld.tile([128, Q], BF16, name=f"qx{b}{ch}", tag=f"qx{b}{ch}")
                    nc.sync.dma_start(out=t[:, :], in_=qxT[b, ch, :, :])
                    qx_all[(b, ch)] = t

            # b0 activations first (kv feeds the first prologue matmuls),
            # then the consts needed by the v evacuations / first tiles.
            for ch in range(2):
                t = ld.tile([128, K], BF16, name=f"kv0{ch}", tag=f"kv0{ch}")
                nc.sync.dma_start(out=t[:, :], in_=kvT[0, ch, :, :])
                kv_all[(0, ch)] = t
            bm_sb = []
            for b in range(B):
                t = pers.tile([128, NKT], F32, name=f"bm_sb{b}")
                nc.sync.dma_start(out=t[:, :], in_=expbm[b, :, :])
                bm_sb.append(t)
            for ch in range(2):
                t = ld.tile([128, Q], BF16, name=f"qx0{ch}", tag=f"qx0{ch}")
                nc.sync.dma_start(out=t[:, :], in_=qxT[0, ch, :, :])
                qx_all[(0, ch)] = t
            for ch in range(2):
                t = pers.tile([128, 128], BF16, name=f"wq_sb{ch}")
                nc.sync.dma_start(out=t[:, :], in_=wq[ch, :, :])
                wq_sb.append(t)
            al_sb = pers.tile([128, 1024], F32, name="al_sb")
            nc.vector.memset(al_sb[:, :], BSHIFT + ALPHA)
            sb_sb = pers.tile([128, 1], F32, name="sb_sb")
            nc.gpsimd.memset(sb_sb[:, :], SCAL_BIAS)
            # ones row at partition 32 (matmul wants lhsT/rhs base aligned:
            # the moving l row lives at partition 32 of posb)
            ones32_sb = pers.tile([D + 1, D], F32, name="ones32_sb")
            nc.gpsimd.memset(ones32_sb[:, :], 1.0)
            id_sb = pers.tile([128, 128], BF16, name="id_sb")
            nc.sync.dma_start(out=id_sb[:, :], in_=ident[:, :])

            # exp(bias_pair) preload: whole head slice in SBUF (64KB/part).
            # qc0 is split so the first tiles' bias lands early.
            ebp_sb = pers.tile([128, NQC, NKT * 512], BF16, name="ebp_sb")
            nc.sync.dma_start(out=ebp_sb[:, 0, 0:8 * 512],
                              in_=ebp[0, :, 0:8, :]
                              .rearrange("p t q -> p (t q)"))
            wobg_sb = pers.tile([D, C + 1], F32R, name="wobg_sb")
            nc.sync.dma_start(out=wobg_sb[:, :], in_=wobg[:, :])
            wo_sb = wobg_sb[:, 0:C]
            bg_sb = wobg_sb[:, C:C + 1].bitcast(F32)
            nc.sync.dma_start(out=ebp_sb[:, 0, 8 * 512:],
                              in_=ebp[0, :, 8:, :]
                              .rearrange("p t q -> p (t q)"))

            def load_ebp(qc):
                nc.sync.dma_start(out=ebp_sb[:, qc, :], in_=ebp[qc, :, :, :]
                                  .rearrange("p t q -> p (t q)"))

            load_ebp(1)
            load_acts(1)
            load_ebp(2)
            load_ebp(3)

            # ---- per-batch projections ----
            from collections import deque
            qT_rep, kT_sb, v_sb, gp_sb = {}, {}, {}, {}

            def prologue_parts(b):
                """Prologue as a list of small closures so b1's projections
                can be drip-fed into b0's main loop (no burst stall)."""
                qx_b = [qx_all[(b, ch)] for ch in range(2)]
                kv_b = [kv_all[(b, ch)] for ch in range(2)]
                qT = pers.tile([128, Q], BF16, name=f"qT{b}")
                gp1 = pers.tile([32, Q], F32, name=f"gp1{b}")
                kT = pers.tile([128, 512], BF16, name=f"kT{b}")
                vt = pers.tile([128, NKT, D + 1], BF16, name=f"v{b}")
                qT_rep[b] = qT; kT_sb[b] = kT; v_sb[b] = vt; gp_sb[b] = gp1
                parts = []

                # k.T in strip layout: strip j (partitions 32j) holds tiles
                # {4g+j} at free cols g*128.  kvT is host-permuted so pos
                # 4j+g holds tile 4g+j -> moving data is contiguous.
                def do_kT():
                    ps = ps_m.tile([128, 512], F32, tag="m", name=f"pk{b}")
                    for ch in range(2):
                        for j in range(4):
                            nc.tensor.matmul(
                                ps[32 * j:32 * j + 32, :], wk_sb[ch][:, :],
                                kv_b[ch][:, j * 512:(j + 1) * 512],
                                start=(ch == 0), stop=(ch == 1),
                                tile_position=(0, 32 * j))
                    nc.vector.tensor_copy(kT[:, :], ps[:, :])
                parts.append(do_kT)

                # v tiles [k-partitions, d] + exp(bm) ones col; the exp(bm)
                # row scale (bias_mask fold: w and l scale alike) is applied
                # during PSUM evacuation via tensor_scalar_mul.
                def make_v(p):
                    def do_v():
                        ps = ps_m.tile([128, 2, D], F32, tag="m",
                                       name=f"pv{b}{p}",
                                       padded_shape=[128, 2, 256])
                        for i in range(2):
                            kt = 2 * p + i
                            pos = 4 * (kt % 4) + kt // 4
                            for ch in range(2):
                                nc.tensor.matmul(
                                    ps[:, i, :],
                                    kv_b[ch][:, pos * 128:(pos + 1) * 128],
                                    wv_sb[ch][:, :], start=(ch == 0),
                                    stop=(ch == 1))
                        for i in range(2):
                            kt = 2 * p + i
                            nc.vector.tensor_scalar_mul(
                                vt[:, kt, 0:D], ps[:, i, :],
                                bm_sb[b][:, kt:kt + 1])
                    return do_v
                parts.extend(make_v(p) for p in range(8))

                # ones columns (scaled by exp(bm)) for the denominator
                parts.append(lambda: nc.vector.tensor_copy(
                    vt[:, :, D:D + 1].rearrange("p k o -> p (k o)"),
                    bm_sb[b][:, :]))

                # q.T replicated into 4 partition strips via 4x-duplicated
                # weight columns (host-prepared); scale folded: sD*log2e*128
                def make_q(qc):
                    def do_q():
                        ps = ps_m.tile([128, 512], F32, tag="m",
                                       name=f"pq{b}{qc}")
                        for ch in range(2):
                            nc.tensor.matmul(
                                ps[:, :], wq_sb[ch][:, :],
                                qx_b[ch][:, qc * 512:(qc + 1) * 512],
                                start=(ch == 0), stop=(ch == 1))
                        nc.scalar.activation(qT[:, qc * 512:(qc + 1) * 512],
                                             ps[:, :], COPY)
                    return do_q
                parts.extend(make_q(qc) for qc in range(NQC))

                # gate: tanh(z/2 + bg/2); (tanh+1)/2 with the 1/2 in Wv
                def make_g(qc):
                    def do_g():
                        ps = ps_m.tile([32, 512], F32, tag="m",
                                       name=f"pg{b}{qc}",
                                       padded_shape=[128, 512])
                        for ch in range(2):
                            nc.tensor.matmul(
                                ps[:, :], wg_sb[ch][:, :],
                                qx_b[ch][:, qc * 512:(qc + 1) * 512],
                                start=(ch == 0), stop=(ch == 1))
                        gt = epool.tile([32, 512], F32, tag="gt",
                                        name=f"gt{b}{qc}")
                        nc.scalar.activation(gt[:, :], ps[:, :],
                                             TANH, bias=bg_sb, scale=0.5)
                        nc.vector.tensor_scalar_add(
                            gp1[:, qc * 512:(qc + 1) * 512], gt[:, :], 1.0)
                    return do_g
                parts.extend(make_g(qc) for qc in range(NQC))
                return parts

            for part in prologue_parts(0):
                part()
            proq = deque(prologue_parts(1))

            # ---- main attention loop (b outer: b1 acts can arrive late) ----
            workq = deque()
            pend_ep2 = [None]

            def make_ep(qc, b, po):
                # po rows 0:32 = o (all 16 k-tiles), row 32 = l.  The gate
                # multiply reads po straight from PSUM on the DVE; only the
                # 1-row l needs an SBUF hop for the PE broadcast matmul.
                # Split in two so the PE-side Wo matmuls trail the division
                # chain by a couple of tiles.
                st = {}

                def ep_part1():
                    lrow = epool.tile([D + 1, 512], F32R, tag="lrow",
                                      name=f"lrow{qc}{b}")
                    nc.vector.tensor_copy(lrow[D:D + 1, :], po[D:D + 1, :])
                    og = epool.tile([D, 512], F32R, tag="og",
                                    name=f"og{qc}{b}")
                    nc.vector.tensor_tensor(
                        og[:, :], po[0:D, :],
                        gp_sb[b][:, qc * 512:(qc + 1) * 512], op=MULT)
                    psl = ps_m.tile([D, 512], F32, tag="m",
                                    name=f"psl{qc}{b}",
                                    padded_shape=[128, 512])
                    nc.tensor.matmul(
                        psl[:, :], ones32_sb[D:D + 1, :].bitcast(F32R),
                        lrow[D:D + 1, :], start=True, stop=True)
                    rlt = epool.tile([D, 512], F32, tag="rl",
                                     name=f"rl{qc}{b}")
                    nc.vector.reciprocal_approx_fast(rlt[:, :], psl[:, :])
                    if kdebug and b == 0 and qc == 0:
                        for ch in range(2):
                            nc.sync.dma_start(
                                out=dbg_ax[ch], in_=kv_all[(0, ch)][:, :])
                            nc.sync.dma_start(
                                out=dbg_ax[2 + ch], in_=qx_all[(0, ch)][:, :])
                        nc.sync.dma_start(out=dbg_eb[:, :, :],
                                          in_=ebp_sb[:, 0, :]
                                          .rearrange("p (t q) -> p t q", q=512))
                        nc.sync.dma_start(out=dbg_po[0:D, :],
                                          in_=og[:, :].bitcast(F32))
                        nc.sync.dma_start(out=dbg_gp[:, :], in_=gp_sb[0][:, :])
                        nc.sync.dma_start(out=dbg_kq[:, 0:512],
                                          in_=kT_sb[0][:, :])
                        nc.sync.dma_start(out=dbg_kq[:, 512:],
                                          in_=qT_rep[0][:, :])
                        nc.sync.dma_start(
                            out=dbg_vt[:, :],
                            in_=v_sb[0][:, :, :].rearrange("p k o -> p (k o)"))
                    st["og"] = og
                    st["rlt"] = rlt

                def ep_part2():
                    og2 = epool.tile([D, 512], F32R, tag="og2",
                                     name=f"og2{qc}{b}")
                    nc.vector.tensor_tensor(og2[:, :], st["og"][:, :],
                                            st["rlt"][:, :], op=MULT)
                    for half in range(2):
                        pp = ps_m.tile([128, 512], F32, tag="m",
                                       name=f"pp{qc}{b}{half}")
                        nc.tensor.matmul(
                            pp[:, :], wo_sb[:, half * 128:(half + 1) * 128],
                            og2[:, :], start=True, stop=True)
                        ot = obpool.tile([128, 512], FP16, tag="ot",
                                         name=f"ot{qc}{b}{half}")
                        if half == 0:
                            nc.vector.tensor_copy(ot[:, :], pp[:, :])
                        else:
                            nc.scalar.activation(ot[:, :], pp[:, :], COPY)
                        nc.sync.dma_start(
                            out=outT[b, half, :, qc * 512:(qc + 1) * 512],
                            in_=ot[:, :])

                return ep_part1, ep_part2

            for b in range(B):
                if b == 1:
                    while proq:
                        proq.popleft()()
                for qc in range(NQC):
                    po = ps_o.tile([128, 512], F32, tag="o", name=f"po{qc}{b}")
                    for g in range(4):
                        for X in range(2):
                            tix = 2 * g + X
                            dve = tix in DVE_TILES
                            has_id = tix in ID_TILES
                            ks = (4 * g + 2 * X) * 512
                            sc = ps_sc.tile([128, 1024], F32, tag="sc",
                                            name=f"s{qc}{b}{g}{X}")
                            if has_id:
                                # bias_pair into PSUM via identity matmuls
                                # (y-domain bias slice); scores accumulate.
                                for jj in range(2):
                                    nc.tensor.matmul(
                                        sc[:, jj * 512:(jj + 1) * 512],
                                        id_sb[:, :],
                                        ebp_sb[:, qc,
                                               ks + jj * 512:ks + (jj + 1) * 512],
                                        start=True, stop=False)
                            # score matmuls interleaved with trailing work
                            # (one queue pop each) so the PE stream stays
                            # dense -> the HAM clock-gate stays at 8/8.
                            for jj in range(2):
                                j = 2 * X + jj
                                nc.tensor.matmul(
                                    sc[:, jj * 512:(jj + 1) * 512],
                                    kT_sb[b][32 * j:32 * j + 32,
                                             g * 128:(g + 1) * 128],
                                    qT_rep[b][32 * j:32 * j + 32,
                                              qc * 512:(qc + 1) * 512],
                                    start=not has_id, stop=True,
                                    tile_position=(32 * j, 0))
                                if workq:
                                    workq.popleft()()
                            wt = wpool.tile([128, 1024], BF16, tag="w",
                                            name=f"w{qc}{b}{g}{X}")
                            if has_id:
                                if dve:
                                    nc.vector._custom_dve(
                                        EXP2, out=wt[:, :].bitcast(I16),
                                        in0=sc[:, :], in1=al_sb[:, :],
                                        s0=MAGIC, s1=EXP_A, imm2=EXP_B)
                                else:
                                    nc.scalar.activation(wt[:, :], sc[:, :],
                                                         EXP, bias=sb_sb[:, :],
                                                         scale=SCAL_SCALE)
                            else:
                                wt0 = w0pool.tile([128, 1024], BF16, tag="w0",
                                                  name=f"w0{qc}{b}{g}{X}")
                                if dve:
                                    nc.vector._custom_dve(
                                        EXP2, out=wt0[:, :].bitcast(I16),
                                        in0=sc[:, :], in1=al_sb[:, :],
                                        s0=MAGIC, s1=EXP_A, imm2=EXP_B)
                                else:
                                    nc.scalar.activation(wt0[:, :], sc[:, :],
                                                         EXP, bias=sb_sb[:, :],
                                                         scale=SCAL_SCALE)
                                nc.vector.tensor_tensor(
                                    wt[:, :], wt0[:, :],
                                    ebp_sb[:, qc, ks:ks + 1024], op=MULT)
                            if (kdebug and b == 0 and qc == 0 and tix == 1
                                    and not has_id):
                                nc.sync.dma_start(out=dbg_wt[0], in_=wt0[:, :])
                                nc.sync.dma_start(out=dbg_wt[1], in_=wt[:, :])
                            while len(workq) > 4:
                                workq.popleft()()
                            # drip-feed b1's prologue through b0's qc1+
                            if b == 0 and qc >= 1 and proq:
                                proq.popleft()()

                            def make_oj(b, g, X, po, wt, i):
                                p = 2 * g + X

                                def emit_o():
                                    kt = 4 * g + 2 * X + i
                                    nc.tensor.matmul(
                                        po[0:D + 1, :], v_sb[b][:, kt, :],
                                        wt[:, i * 512:(i + 1) * 512],
                                        start=(p == 0 and i == 0),
                                        stop=(p == 7 and i == 1))
                                return emit_o
                            workq.append(make_oj(b, g, X, po, wt, 0))
                            workq.append(make_oj(b, g, X, po, wt, 1))
                            if tix == 2 and pend_ep2[0] is not None:
                                workq.append(pend_ep2[0])
                                pend_ep2[0] = None
                            if tix == 7:
                                ep1, ep2 = make_ep(qc, b, po)
                                workq.append(ep1)
                                pend_ep2[0] = ep2

            if pend_ep2[0] is not None:
                workq.append(pend_ep2[0])
                pend_ep2[0] = None
            while workq:
                workq.popleft()()
    nc.compile()
    return nc


def _get_nc():
    if "nc" not in _CACHE:
        _CACHE["nc"] = _build()
    return _CACHE["nc"]


def kernel(q_x, kv_x, bias_mask, bias_pair, Wq, Wk, Wv, Wo, bo, Wg, bg):
    from concourse.bass_utils import run_bass_kernel_spmd

    nc = _get_nc()
    f32 = np.float32
    q_x = np.asarray(q_x, f32); kv_x = np.asarray(kv_x, f32)
    bias_mask = np.asarray(bias_mask, f32); bias_pair = np.asarray(bias_pair, f32)
    Wq = np.asarray(Wq, f32); Wk = np.asarray(Wk, f32); Wv = np.asarray(Wv, f32)
    Wo = np.asarray(Wo, f32); bo = np.asarray(bo, f32); Wg = np.asarray(Wg, f32)
    bg = np.asarray(bg, f32)

    import ml_dtypes
    _bf16 = ml_dtypes.bfloat16
    sD = 1.0 / math.sqrt(D)
    yscale = LOG2E * 128.0
    qxT_dev = np.ascontiguousarray(
        q_x.transpose(0, 2, 1).reshape(B, 2, 128, Q)).astype(_bf16)
    # kvT with k-tiles permuted: position 4j+g holds tile 4g+j, so the
    # k.T strip matmuls read contiguous moving data.
    kv_t = kv_x.transpose(0, 2, 1).reshape(B, 2, 128, NKT, 128)
    perm = [4 * (p % 4) + p // 4 for p in range(NKT)]
    kvT_dev = np.ascontiguousarray(
        kv_t[:, :, :, perm, :].reshape(B, 2, 128, K)).astype(_bf16)
    bm_dev = np.ascontiguousarray(
        np.exp(bias_mask.reshape(B, NKT, 128).transpose(0, 2, 1)))

    def wsplit(W, h, scale=1.0):
        # [2, 128, D] view of (W_h * scale).T with W_h = W[h*D:(h+1)*D, :]
        return np.ascontiguousarray(
            (W[h * D:(h + 1) * D, :] * scale).T.reshape(2, 128, D))

    def wrep(W, h, scale=1.0):
        # weight columns duplicated 4x -> M=128 matmul emits 4 replicas
        wt = wsplit(W, h, scale)                       # [2, 128, D]
        return np.ascontiguousarray(np.tile(wt, (1, 1, 4)))

    in_maps = []
    for h in range(H):
        bp = bias_pair[0, h].T                                 # [K, Q]
        # per k-tile: ID_TILES get the y-domain bias (PE identity-matmul
        # injection), others get exp(bias) (DVE multiply into the weights)
        bpt = bp.reshape(NKT, 128, NQC, 512)
        ebp = np.empty_like(bpt)
        for kt in range(NKT):
            tix = 2 * (kt // 4) + (kt % 4) // 2
            if tix in ID_TILES:
                ebp[kt] = bpt[kt] * yscale
            else:
                ebp[kt] = np.exp(bpt[kt])
        ebp_dev = np.ascontiguousarray(
            ebp.astype(_bf16).transpose(2, 1, 0, 3))
        wkvg_h = np.concatenate(
            [wsplit(Wk, h), wsplit(Wv, h, 0.5), wsplit(Wg, h)],
            axis=2).astype(_bf16)
        wobg_h = np.concatenate(
            [np.ascontiguousarray(Wo[:, h * D:(h + 1) * D].T),
             0.5 * bg[h * D:(h + 1) * D, None]], axis=1)
        in_maps.append({
            "qxT": qxT_dev, "kvT": kvT_dev,
            "ebp": ebp_dev,
            "ident": np.eye(128, dtype=_bf16),
            "expbm": bm_dev,
            "wq": wrep(Wq, h, sD * yscale).astype(_bf16),
            "wkvg": np.ascontiguousarray(wkvg_h),
            "wobg": np.ascontiguousarray(wobg_h.astype(f32)),
        })

    try:
        res = run_bass_kernel_spmd(nc, in_maps, core_ids=list(range(H)))
    except Exception:
        # rare transient accelerator fault — one retry after a short pause
        import time as _time
        _time.sleep(5)
        res = run_bass_kernel_spmd(nc, in_maps, core_ids=list(range(H)))
    out = np.zeros((B, Q, C), f32)
    for h in range(H):
        p = res.results[h]["outT"].astype(f32).reshape(B, C, Q)
        out += p.transpose(0, 2, 1)
    out += bo
    return out
